# revision 1
# baseline (speedup 1.0000x reference)
"""Trainium2 Bass kernel for nn_ChunkedMultiHeadCardPassingLayer.

Sharding: 8 cores = (batch b = core//2) x (T-half = core%2). Each core
processes 2048 contiguous tokens of one batch end-to-end; the only
cross-core dependency is the chunk-carry prefix, resolved with a 4KB
paired AllReduce.

Self-contained: hardcodes shapes; host-side prep is limited to slicing,
transposes and tiny constant matrices.
"""
import os
os.environ.setdefault("JAX_PLATFORMS", "cpu")

import numpy as np
import ml_dtypes
from contextlib import ExitStack

import concourse.bacc as bacc
import concourse.mybir as mybir
import concourse.tile as tile
from concourse.bass_utils import run_bass_kernel_spmd

F32 = mybir.dt.float32
F32R = mybir.dt.float32r
BF16 = mybir.dt.bfloat16
AX = mybir.AxisListType
ALU = mybir.AluOpType
ACTF = mybir.ActivationFunctionType

# problem constants
B, T, C = 4, 4096, 1024
H, CS = 16, 128
D = C // H            # 64
NCORES = 8
R = T // 2            # 2048 rows per core
NCH = R // CS         # 16 chunks per core
NG = C // 128         # 8 groups of (2 heads x 64)
NPG = NCH // 4        # 4 position groups of 512
EPS = 1e-5
P = 128


def _build(ncores, alpha, has_mark_b, has_gate_b, has_proj_b,
           has_carry_gb, has_ln_g, has_ln_b):
    nc = bacc.Bacc("TRN2", target_bir_lowering=False, debug=False,
                   num_devices=ncores)

    # ---------------- DRAM I/O ----------------
    xt_d = nc.dram_tensor("xt", [C, R], F32R, kind="ExternalInput")
    xn_d = nc.dram_tensor("xn", [R, C], F32, kind="ExternalInput")
    mkw_d = nc.dram_tensor("mkw", [C, C], F32R, kind="ExternalInput")
    gtw_d = nc.dram_tensor("gtw", [C, C], F32R, kind="ExternalInput")
    pjw_d = nc.dram_tensor("pjw", [C, C], F32R, kind="ExternalInput")
    mkb_d = nc.dram_tensor("mkb", [1, C], F32R, kind="ExternalInput")
    gtb_d = nc.dram_tensor("gtb", [1, C], F32R, kind="ExternalInput")
    pjb_d = nc.dram_tensor("pjb", [1, C], F32R, kind="ExternalInput")
    w1x_d = nc.dram_tensor("w1x", [2 * D, 2 * D], F32R, kind="ExternalInput")
    w1c_d = nc.dram_tensor("w1c", [2 * D, 2 * D], F32R, kind="ExternalInput")
    b1_d = nc.dram_tensor("b1c", [2 * D, 1], F32, kind="ExternalInput")
    w2_d = nc.dram_tensor("w2", [2 * D, D], F32R, kind="ExternalInput")
    b2_d = nc.dram_tensor("b2c", [D, 1], F32, kind="ExternalInput")
    ut_d = nc.dram_tensor("ut", [P, P], F32R, kind="ExternalInput")
    st_d = nc.dram_tensor("st", [P, P], BF16, kind="ExternalInput")
    l0_d = nc.dram_tensor("l0", [NCH, NCH], F32, kind="ExternalInput")
    eye_d = nc.dram_tensor("eyer", [P, P], F32R, kind="ExternalInput")
    csel_d = nc.dram_tensor("csel", [P, NCH * NCH], F32R,
                            kind="ExternalInput")
    onesr_d = nc.dram_tensor("onesr", [1, P], F32R, kind="ExternalInput")
    segm_d = nc.dram_tensor("segm", [1, 1], F32, kind="ExternalInput")
    usem_d = nc.dram_tensor("usem", [1, 1], F32, kind="ExternalInput")
    cgr_d = nc.dram_tensor("cgr", [NCH, D], F32, kind="ExternalInput")
    cbr_d = nc.dram_tensor("cbr", [NCH, D], F32, kind="ExternalInput")
    lgr_d = nc.dram_tensor("lgr", [P, C], F32, kind="ExternalInput")
    lbr_d = nc.dram_tensor("lbr", [P, C], F32, kind="ExternalInput")

    y_d = nc.dram_tensor("y", [R, C], F32, kind="ExternalOutput")

    lc_d = nc.dram_tensor("lc_spill", [R, C], BF16)   # local_cum spill
    cc_in = nc.dram_tensor("cc_in", [1, C], F32)
    cc_out = nc.dram_tensor("cc_out", [1, C], F32)

    groups = ([[i, i + 1] for i in range(0, ncores, 2)]
              if ncores > 1 else [[0]])

    with tile.TileContext(nc) as tc, ExitStack() as top:
        const_p = top.enter_context(tc.tile_pool(name="const", bufs=1))
        carr_p = top.enter_context(tc.tile_pool(name="carr", bufs=1))

        # ---------- constants ----------
        ut = const_p.tile([P, P], F32R)
        st = const_p.tile([P, P], BF16)
        l0 = const_p.tile([NCH, NCH], F32)
        eyer = const_p.tile([P, P], F32R)
        csel = const_p.tile([P, NCH * NCH], F32R)
        w1x = const_p.tile([2 * D, 2 * D], F32R)
        w1c = const_p.tile([2 * D, 2 * D], F32R)
        b1c = const_p.tile([2 * D, 1], F32)
        w2 = const_p.tile([2 * D, D], F32R)
        b2c = const_p.tile([D, 1], F32)
        segm = const_p.tile([1, 1], F32)
        usem = const_p.tile([1, 1], F32)
        for t_, d_ in ((ut, ut_d), (st, st_d), (l0, l0_d), (eyer, eye_d),
                       (csel, csel_d), (w1x, w1x_d), (w1c, w1c_d),
                       (b1c, b1_d), (w2, w2_d), (b2c, b2_d), (segm, segm_d),
                       (usem, usem_d)):
            nc.sync.dma_start(t_[:], d_.ap())
        ones1r = const_p.tile([1, P], F32R)
        nc.sync.dma_start(ones1r[:], onesr_d.ap())
        ones1_16 = const_p.tile([1, NCH], F32)
        nc.vector.memset(ones1_16[:], 1.0)
        ones16_1 = const_p.tile([NCH, 1], F32)
        nc.vector.memset(ones16_1[:], 1.0)
        eps128 = const_p.tile([P, 1], F32)
        nc.vector.memset(eps128[:], EPS)
        if has_mark_b or has_gate_b or has_proj_b:
            mkb = const_p.tile([1, C], F32R)
            gtb = const_p.tile([1, C], F32R)
            pjb = const_p.tile([1, C], F32R)
            nc.sync.dma_start(mkb[:], mkb_d.ap())
            nc.sync.dma_start(gtb[:], gtb_d.ap())
            nc.sync.dma_start(pjb[:], pjb_d.ap())
        if has_carry_gb:
            cgr = const_p.tile([NCH, D], F32)
            cbr = const_p.tile([NCH, D], F32)
            nc.sync.dma_start(cgr[:], cgr_d.ap())
            nc.sync.dma_start(cbr[:], cbr_d.ap())

        cs_sb = carr_p.tile([NCH, C], F32)
        ncarry = carr_p.tile([NCH, C], F32R)

        # ================ phase 1: pm/gate/scan ================
        with tc.tile_pool(name="xtp", bufs=1) as xt_p, \
             tc.tile_pool(name="wgt", bufs=1) as wgt_p, \
             tc.tile_pool(name="ph1", bufs=3) as ph1_p, \
             tc.tile_pool(name="ps1", bufs=2, space="PSUM") as ps1_p, \
             tc.tile_pool(name="pslc", bufs=2, space="PSUM") as pslc_p, \
             tc.tile_pool(name="pscs", bufs=1, space="PSUM") as pscs_p:
            xt = []
            for g in range(NG):
                t_ = xt_p.tile([P, R], F32R, tag=f"xt{g}", name=f"xt{g}")
                nc.sync.dma_start(t_[:], xt_d.ap()[g * P:(g + 1) * P, :])
                xt.append(t_)
            mkw, gtw = [], []
            for k in range(NG):
                mt = wgt_p.tile([P, C], F32R, tag=f"mk{k}", name=f"mk{k}")
                gt_ = wgt_p.tile([P, C], F32R, tag=f"gk{k}", name=f"gk{k}")
                nc.sync.dma_start(mt[:], mkw_d.ap()[k * P:(k + 1) * P, :])
                nc.sync.dma_start(gt_[:], gtw_d.ap()[k * P:(k + 1) * P, :])
                mkw.append(mt)
                gtw.append(gt_)

            cs_ps = pscs_p.tile([NCH, C], F32, tag="csps")
            for j in range(NCH):
                for n in range(2):
                    sl = slice(n * 512, (n + 1) * 512)
                    pm_ps = ps1_p.tile([P, 512], F32, tag="pm", name="pm_ps")
                    gt_ps = ps1_p.tile([P, 512], F32, tag="gt", name="gt_ps")
                    for k in range(NG):
                        lhs = xt[k][:, j * P:(j + 1) * P]
                        st_ = (k == 0)
                        sp = (k == NG - 1) and not (has_mark_b or has_gate_b)
                        nc.tensor.matmul(pm_ps[:], lhs, mkw[k][:, sl],
                                         start=st_, stop=sp)
                        nc.tensor.matmul(gt_ps[:], lhs, gtw[k][:, sl],
                                         start=st_, stop=sp)
                    if has_mark_b or has_gate_b:
                        nc.tensor.matmul(pm_ps[:], ones1r[:], mkb[:, sl],
                                         start=False, stop=True)
                        nc.tensor.matmul(gt_ps[:], ones1r[:], gtb[:, sl],
                                         start=False, stop=True)
                    gates = ph1_p.tile([P, 512], F32, tag="gates",
                                       name="gates")
                    nc.scalar.activation(gates[:], gt_ps[:], ACTF.Sigmoid)
                    gated = ph1_p.tile([P, 512], F32R, tag="gated",
                                       name="gated")
                    nc.vector.tensor_tensor(gated[:], gates[:], pm_ps[:],
                                            op=ALU.mult)
                    nc.tensor.matmul(cs_ps[:, sl],
                                     csel[:, j * NCH:(j + 1) * NCH],
                                     gated[:], start=(j == 0),
                                     stop=(j == NCH - 1))
                    lc_ps = pslc_p.tile([P, 512], F32, tag="lcps",
                                        name="lc_ps")
                    nc.tensor.matmul(lc_ps[:], ut[:], gated[:],
                                     start=True, stop=True)
                    lcs = ph1_p.tile([P, 512], BF16, tag="lcs", name="lcs")
                    nc.vector.tensor_copy(lcs[:], lc_ps[:])
                    nc.sync.dma_start(
                        lc_d.ap()[j * P:(j + 1) * P, sl], lcs[:])
            nc.vector.tensor_copy(cs_sb[:], cs_ps[:])

        # ================ carries + collective ================
        with tc.tile_pool(name="car", bufs=1) as car_p, \
             tc.tile_pool(name="pscar", bufs=1, space="PSUM") as pscar_p:
            tot_ps = pscar_p.tile([1, C], F32, tag="tot")
            for n in range(2):
                sl = slice(n * 512, (n + 1) * 512)
                nc.tensor.matmul(tot_ps[:, sl], ones16_1[:], cs_sb[:, sl],
                                 start=True, stop=True)
            ccin_sb = car_p.tile([1, C], F32)
            nc.vector.tensor_scalar(ccin_sb[:], tot_ps[:], segm[:], None,
                                    op0=ALU.mult)
            nc.sync.dma_start(cc_in.ap(), ccin_sb[:])
            nc.gpsimd.collective_compute(
                "AllReduce", ALU.add, replica_groups=groups,
                ins=[cc_in.ap()], outs=[cc_out.ap()])
            base_sb = car_p.tile([1, C], F32)
            nc.sync.dma_start(base_sb[:], cc_out.ap())
            basem = car_p.tile([1, C], F32)
            nc.vector.tensor_scalar(basem[:], base_sb[:], usem[:], None,
                                    op0=ALU.mult)

            carx_ps = pscar_p.tile([NCH, C], F32, tag="carx")
            for n in range(2):
                sl = slice(n * 512, (n + 1) * 512)
                nc.tensor.matmul(carx_ps[:, sl], l0[:], cs_sb[:, sl],
                                 start=True, stop=False)
                nc.tensor.matmul(carx_ps[:, sl], ones1_16[:], basem[:, sl],
                                 start=False, stop=True)

            # ncarry = LN(carries) over d segments
            carr = car_p.tile([NCH, C], F32)
            nc.vector.tensor_copy(carr[:], carx_ps[:])
            c3 = carr[:].rearrange("p (h d) -> p h d", d=D)
            r1 = car_p.tile([NCH, H], F32)
            nc.vector.tensor_reduce(r1[:], c3, axis=AX.X, op=ALU.add)
            sqc = car_p.tile([NCH, C], F32)
            nc.vector.tensor_tensor(sqc[:], carr[:], carr[:], op=ALU.mult)
            r2 = car_p.tile([NCH, H], F32)
            nc.vector.tensor_reduce(r2[:], sqc[:].rearrange(
                "p (h d) -> p h d", d=D), axis=AX.X, op=ALU.add)
            mu = car_p.tile([NCH, H], F32)
            nc.vector.tensor_scalar(mu[:], r1[:], 1.0 / D, None, op0=ALU.mult)
            em2 = car_p.tile([NCH, H], F32)
            nc.vector.tensor_scalar(em2[:], r2[:], 1.0 / D, None,
                                    op0=ALU.mult)
            musq = car_p.tile([NCH, H], F32)
            nc.vector.tensor_tensor(musq[:], mu[:], mu[:], op=ALU.mult)
            var = car_p.tile([NCH, H], F32)
            nc.vector.tensor_tensor(var[:], em2[:], musq[:], op=ALU.subtract)
            eps16 = car_p.tile([NCH, 1], F32)
            nc.vector.memset(eps16[:], EPS)
            sd = car_p.tile([NCH, H], F32)
            nc.scalar.activation(sd[:], var[:], ACTF.Sqrt, bias=eps16[:])
            rstd = car_p.tile([NCH, H], F32)
            nc.vector.reciprocal(rstd[:], sd[:])
            mu_b = mu[:].unsqueeze(2).to_broadcast([NCH, H, D])
            rstd_b = rstd[:].unsqueeze(2).to_broadcast([NCH, H, D])
            cen = car_p.tile([NCH, C], F32)
            nc.vector.tensor_tensor(cen[:].rearrange("p (h d) -> p h d", d=D),
                                    c3, mu_b, op=ALU.subtract)
            if has_carry_gb:
                nrm = car_p.tile([NCH, C], F32)
                nc.vector.tensor_tensor(
                    nrm[:].rearrange("p (h d) -> p h d", d=D),
                    cen[:].rearrange("p (h d) -> p h d", d=D), rstd_b,
                    op=ALU.mult)
                cg_b = cgr[:].unsqueeze(1).to_broadcast([NCH, H, D])
                cb_b = cbr[:].unsqueeze(1).to_broadcast([NCH, H, D])
                nrm2 = car_p.tile([NCH, C], F32)
                nc.vector.tensor_tensor(
                    nrm2[:].rearrange("p (h d) -> p h d", d=D),
                    nrm[:].rearrange("p (h d) -> p h d", d=D), cg_b,
                    op=ALU.mult)
                nc.vector.tensor_tensor(
                    ncarry[:].rearrange("p (h d) -> p h d", d=D),
                    nrm2[:].rearrange("p (h d) -> p h d", d=D), cb_b,
                    op=ALU.add)
            else:
                nc.vector.tensor_tensor(
                    ncarry[:].rearrange("p (h d) -> p h d", d=D),
                    cen[:].rearrange("p (h d) -> p h d", d=D), rstd_b,
                    op=ALU.mult)

        # ===== phases 2-4, interleaved per position-group of 4 chunks =====
        HH = H // 2  # heads per column half
        with ExitStack() as late:
            pj_p = late.enter_context(tc.tile_pool(name="pjp", bufs=1))
            big_p = late.enter_context(tc.tile_pool(name="bigpool", bufs=28))
            xts_p = late.enter_context(tc.tile_pool(name="xts", bufs=2))
            lcin_p = late.enter_context(tc.tile_pool(name="lcin", bufs=3))
            ph2_p = late.enter_context(tc.tile_pool(name="ph2", bufs=2))
            ph3_p = late.enter_context(tc.tile_pool(name="ph3", bufs=2))
            ph4_p = late.enter_context(tc.tile_pool(name="ph4", bufs=2))
            ps2_p = late.enter_context(
                tc.tile_pool(name="ps2", bufs=2, space="PSUM"))
            pstr_p = late.enter_context(
                tc.tile_pool(name="pstr", bufs=1, space="PSUM"))
            ps3_p = late.enter_context(
                tc.tile_pool(name="ps3", bufs=2, space="PSUM"))
            ps3b_p = late.enter_context(
                tc.tile_pool(name="ps3b", bufs=1, space="PSUM"))
            ps4_p = late.enter_context(
                tc.tile_pool(name="ps4", bufs=1, space="PSUM"))

            pjw = []
            for k in range(NG):
                pt = pj_p.tile([P, C], F32R, tag=f"pj{k}", name=f"pj{k}")
                nc.sync.dma_start(pt[:], pjw_d.ap()[k * P:(k + 1) * P, :])
                pjw.append(pt)
            if has_ln_g:
                lgr = pj_p.tile([P, C], F32)
                nc.sync.dma_start(lgr[:], lgr_d.ap())
            if has_ln_b:
                lbr = pj_p.tile([P, C], F32)
                nc.sync.dma_start(lbr[:], lbr_d.ap())

            for pg in range(NPG):
                psl = slice(pg * 512, (pg + 1) * 512)
                cardsT = [None] * NG
                outT = [None] * NG
                xtg = []
                for g in range(NG):
                    xg = xts_p.tile([P, 512], F32R, tag=f"xts{g}",
                                    name=f"xts{g}")
                    nc.sync.dma_start(
                        xg[:], xt_d.ap()[g * P:(g + 1) * P, psl])
                    xtg.append(xg)
                    cardsT[g] = big_p.tile([P, 512], F32R, tag="bigtile",
                                           name=f"cardsT{pg}_{g}")

                # ---- phase 2: cards for the 4 chunks of this pg ----
                for jj in range(4):
                    j = pg * 4 + jj
                    ncrow = lcin_p.tile([1, C], F32R, tag="ncrow",
                                        name="ncrow", bufs=2)
                    nc.sync.dma_start(ncrow[:], ncarry[j:j + 1, :])
                    for n in range(2):
                        sl = slice(n * 512, (n + 1) * 512)
                        lcj = lcin_p.tile([P, 512], BF16, tag="lcin",
                                          name="lcj", bufs=4)
                        nc.sync.dma_start(lcj[:],
                                          lc_d.ap()[j * P:(j + 1) * P, sl])
                        cl_ps = ps2_p.tile([P, 512], F32, tag="clps",
                                           name="cl_ps")
                        nc.tensor.matmul(cl_ps[:], st[:], lcj[:],
                                         start=True, stop=False)
                        nc.tensor.matmul(cl_ps[:], ones1r[:],
                                         ncrow[0:1, sl],
                                         start=False, stop=True)
                        # segmented LN over d (gamma/beta folded into W1c)
                        sq = ph2_p.tile([P, 512], F32, tag="sq", name="sq")
                        nc.scalar.square(sq[:], cl_ps[:])
                        r1c = ph2_p.tile([P, HH], F32, tag="r1c", name="r1c")
                        nc.vector.tensor_reduce(
                            r1c[:],
                            cl_ps[:].rearrange("p (h d) -> p h d", d=D),
                            axis=AX.X, op=ALU.add)
                        r2c = ph2_p.tile([P, HH], F32, tag="r2c", name="r2c")
                        nc.vector.tensor_reduce(
                            r2c[:], sq[:].rearrange("p (h d) -> p h d", d=D),
                            axis=AX.X, op=ALU.add)
                        muc = ph2_p.tile([P, HH], F32, tag="muc", name="muc")
                        nc.vector.tensor_scalar(muc[:], r1c[:], 1.0 / D,
                                                None, op0=ALU.mult)
                        em2c = ph2_p.tile([P, HH], F32, tag="em2c",
                                          name="em2c")
                        nc.vector.tensor_scalar(em2c[:], r2c[:], 1.0 / D,
                                                None, op0=ALU.mult)
                        musqc = ph2_p.tile([P, HH], F32, tag="musqc",
                                           name="musqc")
                        nc.vector.tensor_tensor(musqc[:], muc[:], muc[:],
                                                op=ALU.mult)
                        varc = ph2_p.tile([P, HH], F32, tag="varc",
                                          name="varc")
                        nc.vector.tensor_tensor(varc[:], em2c[:], musqc[:],
                                                op=ALU.subtract)
                        sdc = ph2_p.tile([P, HH], F32, tag="sdc", name="sdc")
                        nc.scalar.activation(sdc[:], varc[:], ACTF.Sqrt,
                                             bias=eps128[:])
                        rstdc = ph2_p.tile([P, HH], F32, tag="rstdc",
                                           name="rstdc")
                        nc.vector.reciprocal(rstdc[:], sdc[:])
                        mu_bc = muc[:].unsqueeze(2).to_broadcast([P, HH, D])
                        rstd_bc = rstdc[:].unsqueeze(2).to_broadcast(
                            [P, HH, D])
                        cenc = ph2_p.tile([P, 512], F32, tag="cenc",
                                          name="cenc")
                        nc.vector.tensor_tensor(
                            cenc[:].rearrange("p (h d) -> p h d", d=D),
                            cl_ps[:].rearrange("p (h d) -> p h d", d=D),
                            mu_bc, op=ALU.subtract)
                        cards = ph2_p.tile([P, 512], F32R, tag="cards",
                                           name="cards")
                        nc.vector.tensor_tensor(
                            cards[:].rearrange("p (h d) -> p h d", d=D),
                            cenc[:].rearrange("p (h d) -> p h d", d=D),
                            rstd_bc, op=ALU.mult)
                        for gg in range(4):
                            g = n * 4 + gg
                            tr_ps = pstr_p.tile([P, P], F32R, tag="trps",
                                                name="tr_ps")
                            nc.tensor.transpose(
                                tr_ps[:], cards[:, gg * P:(gg + 1) * P],
                                eyer[:])
                            nc.scalar.copy(cardsT[g][:, jj * P:(jj + 1) * P],
                                           tr_ps[:])

                # ---- phase 3: head MLP for this pg ----
                for g in range(NG):
                    outT[g] = big_p.tile([P, 512], F32R, tag="bigtile",
                                         name=f"outT{pg}_{g}")
                for h in range(H):
                    g, off = h // 2, (h % 2) * D
                    h1_ps = ps3_p.tile([P, 512], F32, tag="h1", name="h1_ps")
                    nc.tensor.matmul(h1_ps[:], w1x[off:off + D, :],
                                     xtg[g][off:off + D, :],
                                     start=True, stop=False)
                    nc.tensor.matmul(h1_ps[:], w1c[off:off + D, :],
                                     cardsT[g][off:off + D, :],
                                     start=False, stop=True)
                    sq3 = ph3_p.tile([P, 512], F32, tag="sq3", name="sq3")
                    nc.scalar.activation(sq3[:], h1_ps[:], ACTF.Square,
                                         bias=b1c[:])
                    e3 = ph3_p.tile([P, 512], F32, tag="e3", name="e3")
                    nc.scalar.activation(e3[:], sq3[:], ACTF.Exp, scale=-0.5)
                    hb = ph3_p.tile([P, 512], F32, tag="hb", name="hb")
                    nc.vector.tensor_scalar(hb[:], h1_ps[:], b1c[:], None,
                                            op0=ALU.add)
                    t1 = ph3_p.tile([P, 512], F32, tag="t1", name="t1")
                    nc.vector.tensor_scalar(t1[:], e3[:], float(alpha), 1.0,
                                            op0=ALU.mult, op1=ALU.add)
                    hf = ph3_p.tile([P, 512], F32R, tag="hf", name="hf")
                    nc.vector.tensor_tensor(hf[:], t1[:], hb[:], op=ALU.mult)
                    o2_ps = ps3b_p.tile([D, 512], F32, tag="o2", name="o2_ps")
                    nc.tensor.matmul(o2_ps[:], w2[:], hf[:],
                                     start=True, stop=True)
                    nc.scalar.activation(outT[g][off:off + D, :], o2_ps[:],
                                         ACTF.Identity, bias=b2c[:])

                # ---- phase 4: proj + LN + residual for this pg ----
                for tt in range(4):
                    t_i = pg * 4 + tt
                    col = tt * P
                    y_ps = ps4_p.tile([P, C], F32, tag="yps", name="y_ps")
                    for k in range(NG):
                        lhs = outT[k][:, col:col + P]
                        st_ = (k == 0)
                        sp = (k == NG - 1) and not has_proj_b
                        for n in range(2):
                            sl = slice(n * 512, (n + 1) * 512)
                            nc.tensor.matmul(y_ps[:, sl], lhs, pjw[k][:, sl],
                                             start=st_, stop=sp)
                    if has_proj_b:
                        for n in range(2):
                            sl = slice(n * 512, (n + 1) * 512)
                            nc.tensor.matmul(y_ps[:, sl], ones1r[:],
                                             pjb[:, sl],
                                             start=False, stop=True)
                    y_raw = ph4_p.tile([P, C], F32, tag="yraw", name="y_raw")
                    s1 = ph4_p.tile([P, 1], F32, tag="s1", name="s1")
                    nc.scalar.activation(y_raw[:], y_ps[:], ACTF.Copy,
                                         accum_out=s1[:])
                    sc4 = ph4_p.tile([P, C], F32, tag="sc4", name="sc4",
                                     bufs=1)
                    s2 = ph4_p.tile([P, 1], F32, tag="s2", name="s2")
                    nc.scalar.activation(sc4[:], y_ps[:], ACTF.Square,
                                         scale=1.0 / 32.0, accum_out=s2[:])
                    m1 = ph4_p.tile([P, 1], F32, tag="m1", name="m1")
                    nc.vector.tensor_scalar(m1[:], s1[:], 1.0 / C, None,
                                            op0=ALU.mult)
                    msq = ph4_p.tile([P, 1], F32, tag="msq", name="msq")
                    nc.vector.tensor_tensor(msq[:], m1[:], m1[:],
                                            op=ALU.mult)
                    var4 = ph4_p.tile([P, 1], F32, tag="var4", name="var4")
                    nc.vector.tensor_tensor(var4[:], s2[:], msq[:],
                                            op=ALU.subtract)
                    sd4 = ph4_p.tile([P, 1], F32, tag="sd4", name="sd4")
                    nc.scalar.activation(sd4[:], var4[:], ACTF.Sqrt,
                                         bias=eps128[:])
                    rstd4 = ph4_p.tile([P, 1], F32, tag="rstd4",
                                       name="rstd4")
                    nc.vector.reciprocal(rstd4[:], sd4[:])
                    tnorm = ph4_p.tile([P, C], F32, tag="tnorm",
                                       name="tnorm")
                    nc.vector.tensor_scalar(tnorm[:], y_raw[:], m1[:],
                                            rstd4[:], op0=ALU.subtract,
                                            op1=ALU.mult)
                    if has_ln_g:
                        nc.vector.tensor_tensor(tnorm[:], tnorm[:], lgr[:],
                                                op=ALU.mult)
                    if has_ln_b:
                        nc.vector.tensor_tensor(tnorm[:], tnorm[:], lbr[:],
                                                op=ALU.add)
                    xa = ph4_p.tile([P, C], F32, tag="xa", name="xa")
                    nc.sync.dma_start(xa[:],
                                      xn_d.ap()[t_i * P:(t_i + 1) * P, :])
                    nc.vector.tensor_tensor(tnorm[:], tnorm[:], xa[:],
                                            op=ALU.add)
                    nc.sync.dma_start(y_d.ap()[t_i * P:(t_i + 1) * P, :],
                                      tnorm[:])

    nc.compile()
    return nc


_CACHE = {}


def _get_program(alpha, flags):
    key = (alpha, flags)
    if key not in _CACHE:
        _CACHE[key] = _build(NCORES, alpha, *flags)
    return _CACHE[key]


def make_consts(W1, b1, card_g, card_b, carry_g, carry_b, ln_g, ln_b):
    W1x = np.ascontiguousarray(np.concatenate([W1[:D, :], W1[:D, :]], 0))
    W1c0 = card_g[:, None] * W1[D:, :]
    W1c = np.ascontiguousarray(np.concatenate([W1c0, W1c0], 0))
    b1f = (b1 + card_b @ W1[D:, :]).astype(np.float32)
    ut = np.triu(np.ones((P, P), np.float32))
    stm = np.zeros((P, P), np.float32)
    for i in range(1, P):
        stm[i - 1, i] = 1.0
    stm = stm.astype(ml_dtypes.bfloat16)
    l0 = np.triu(np.ones((NCH, NCH), np.float32), k=1)
    csel = np.zeros((P, NCH, NCH), np.float32)
    for j in range(NCH):
        csel[:, j, j] = 1.0
    csel = csel.reshape(P, NCH * NCH)
    return {
        "w1x": W1x, "w1c": W1c, "b1c": b1f[:, None],
        "ut": ut, "st": stm, "l0": l0, "csel": csel,
        "eyer": np.eye(P, dtype=np.float32),
        "onesr": np.ones((1, P), np.float32),
        "cgr": np.tile(carry_g[None, :], (NCH, 1)).astype(np.float32),
        "cbr": np.tile(carry_b[None, :], (NCH, 1)).astype(np.float32),
        "lgr": np.tile(ln_g[None, :], (P, 1)).astype(np.float32),
        "lbr": np.tile(ln_b[None, :], (P, 1)).astype(np.float32),
    }


def kernel(**inputs):
    x = np.ascontiguousarray(np.asarray(inputs["x"], np.float32))
    mark_W = np.asarray(inputs["mark_W"], np.float32)
    mark_b = np.asarray(inputs["mark_b"], np.float32)
    gate_W = np.asarray(inputs["gate_W"], np.float32)
    gate_b = np.asarray(inputs["gate_b"], np.float32)
    carry_g = np.asarray(inputs["carry_g"], np.float32)
    carry_b = np.asarray(inputs["carry_b"], np.float32)
    card_g = np.asarray(inputs["card_g"], np.float32)
    card_b = np.asarray(inputs["card_b"], np.float32)
    W1 = np.asarray(inputs["W1"], np.float32)
    b1 = np.asarray(inputs["b1"], np.float32)
    alpha = float(np.asarray(inputs["alpha"]))
    W2 = np.asarray(inputs["W2"], np.float32)
    b2 = np.asarray(inputs["b2"], np.float32)
    proj_W = np.asarray(inputs["proj_W"], np.float32)
    proj_b = np.asarray(inputs["proj_b"], np.float32)
    ln_g = np.asarray(inputs["ln_g"], np.float32)
    ln_b = np.asarray(inputs["ln_b"], np.float32)

    has_carry_gb = bool(np.any(carry_g != 1.0) or np.any(carry_b != 0.0))
    flags = (bool(np.any(mark_b)), bool(np.any(gate_b)), bool(np.any(proj_b)),
             has_carry_gb, bool(np.any(ln_g != 1.0)), bool(np.any(ln_b)))
    nc = _get_program(alpha, flags)

    common = make_consts(W1, b1, card_g, card_b, carry_g, carry_b, ln_g, ln_b)
    common.update({
        "mkw": mark_W, "gtw": gate_W, "pjw": proj_W,
        "mkb": mark_b[None, :], "gtb": gate_b[None, :],
        "pjb": proj_b[None, :],
        "w2": W2, "b2c": b2[:, None],
    })
    in_maps = []
    for c in range(NCORES):
        b, half = c // 2, c % 2
        xs = x[b, half * R:(half + 1) * R, :]
        m = dict(common)
        m["xn"] = np.ascontiguousarray(xs)
        m["xt"] = np.ascontiguousarray(xs.T)
        m["segm"] = np.array([[1.0 - half]], np.float32)
        m["usem"] = np.array([[float(half)]], np.float32)
        in_maps.append(m)

    res = run_bass_kernel_spmd(nc, in_maps, list(range(NCORES)))
    out = np.empty((B, T, C), np.float32)
    for c in range(NCORES):
        b, half = c // 2, c % 2
        out[b, half * R:(half + 1) * R, :] = res.results[c]["y"]
    return out



# revision 39
# speedup vs baseline: 1.0165x; 1.0165x over previous
"""Trainium2 Bass kernel for nn_ChunkedMultiHeadCardPassingLayer.

Sharding: 8 cores = (batch b = core//2) x (T-half = core%2). Each core
processes 2048 contiguous tokens of one batch end-to-end; the only
cross-core dependency is the chunk-carry prefix, resolved with a 4KB
paired AllReduce.

v2 restructure vs baseline:
- all matmul stationaries are 2-byte (bf16) -> cheap LDWEIGHTS
- local_cum kept in SBUF as bf16 (no DRAM spill round-trip)
- chunk sums extracted from cumsum row 127 (csel matmul dropped)
- cards transposed via XBAR DMA-transpose (no PE transposes, no PSUM)
- MLP activation chain spread across scalar/vector/gpsimd engines
- b1/b2 folded into downstream biases; phase pipeline interleaved
"""
import os
os.environ.setdefault("JAX_PLATFORMS", "cpu")

import numpy as np
import ml_dtypes
from contextlib import ExitStack

import concourse.bacc as bacc
import concourse.mybir as mybir
import concourse.tile as tile
from concourse.bass_utils import run_bass_kernel_spmd

F32 = mybir.dt.float32
F32R = mybir.dt.float32r
BF16 = mybir.dt.bfloat16
AX = mybir.AxisListType
ALU = mybir.AluOpType
ACTF = mybir.ActivationFunctionType

# problem constants
B, T, C = 4, 4096, 1024
H, CS = 16, 128
D = C // H            # 64
NCORES = 8
R = T // 2            # 2048 rows per core
NCH = R // CS         # 16 chunks per core
NG = C // 128         # 8 groups of (2 heads x 64)
NPG = NCH // 4        # 4 position groups of 512
EPS = 1e-5
P = 128
HH = 8                # heads per 512 half


def _build(ncores, alpha, has_mkb, has_gtb, has_pjb,
           has_carry_gb, has_ln_g, has_ln_b):
    nc = bacc.Bacc("TRN2", target_bir_lowering=False, debug=False,
                   num_devices=ncores)

    # ---------------- DRAM I/O ----------------
    xt_d = nc.dram_tensor("xt", [C, R], BF16, kind="ExternalInput")
    xn_d = nc.dram_tensor("xn", [R, C], F32, kind="ExternalInput")
    mkw_d = nc.dram_tensor("mkw", [C, C], BF16, kind="ExternalInput")
    gtw_d = nc.dram_tensor("gtw", [C, C], BF16, kind="ExternalInput")
    pjw_d = nc.dram_tensor("pjw", [C, C], BF16, kind="ExternalInput")
    mkb_d = nc.dram_tensor("mkb", [1, C], BF16, kind="ExternalInput")
    gtb_d = nc.dram_tensor("gtb", [1, C], BF16, kind="ExternalInput")
    pjb_d = nc.dram_tensor("pjb", [1, C], BF16, kind="ExternalInput")
    w1x_d = nc.dram_tensor("w1x", [2 * D, 2 * D], BF16, kind="ExternalInput")
    w1c_d = nc.dram_tensor("w1c", [2 * D, 2 * D], BF16, kind="ExternalInput")
    b1_d = nc.dram_tensor("b1c", [2 * D, 1], F32, kind="ExternalInput")
    w2_d = nc.dram_tensor("w2", [2 * D, D], BF16, kind="ExternalInput")
    ut_d = nc.dram_tensor("ut", [P, P], BF16, kind="ExternalInput")
    st_d = nc.dram_tensor("st", [P, P], BF16, kind="ExternalInput")
    l0_d = nc.dram_tensor("l0", [NCH, NCH], BF16, kind="ExternalInput")
    onesr_d = nc.dram_tensor("onesr", [1, P], BF16, kind="ExternalInput")
    selb_d = nc.dram_tensor("selb", [NCH, NCH * P], BF16,
                            kind="ExternalInput")
    segm_d = nc.dram_tensor("segm", [1, 1], F32, kind="ExternalInput")
    usem_d = nc.dram_tensor("usem", [1, 1], F32, kind="ExternalInput")
    cgr_d = nc.dram_tensor("cgr", [NCH, D], F32, kind="ExternalInput")
    cbr_d = nc.dram_tensor("cbr", [NCH, D], F32, kind="ExternalInput")
    lgr_d = nc.dram_tensor("lgr", [P, C], F32, kind="ExternalInput")
    lbr_d = nc.dram_tensor("lbr", [P, C], F32, kind="ExternalInput")

    y_d = nc.dram_tensor("y", [R, C], F32, kind="ExternalOutput")

    cc_in = nc.dram_tensor("cc_in", [1, C], F32)
    cc_out = nc.dram_tensor("cc_out", [1, C], F32)

    groups = ([[i, i + 1] for i in range(0, ncores, 2)]
              if ncores > 1 else [[0]])

    with tile.TileContext(nc) as tc, ExitStack() as top:
        const_p = top.enter_context(tc.tile_pool(name="const", bufs=1))
        xt_p = top.enter_context(tc.tile_pool(name="xtp", bufs=1))
        lc_p = top.enter_context(tc.tile_pool(name="lcp", bufs=1))
        carr_p = top.enter_context(tc.tile_pool(name="carr", bufs=1))

        # ---------- constants ----------
        ut = const_p.tile([P, P], BF16)
        st = const_p.tile([P, P], BF16)
        l0 = const_p.tile([NCH, NCH], BF16)
        w1x = const_p.tile([2 * D, 2 * D], BF16)
        w1c = const_p.tile([2 * D, 2 * D], BF16)
        b1c = const_p.tile([2 * D, 1], F32)
        w2 = const_p.tile([2 * D, D], BF16)
        segm = const_p.tile([1, 1], F32)
        usem = const_p.tile([1, 1], F32)
        ones1r = const_p.tile([1, P], BF16)
        selb = const_p.tile([NCH, NCH * P], BF16)
        for t_, d_ in ((ut, ut_d), (st, st_d), (l0, l0_d),
                       (w1x, w1x_d), (w1c, w1c_d), (b1c, b1_d),
                       (w2, w2_d), (segm, segm_d), (usem, usem_d),
                       (ones1r, onesr_d), (selb, selb_d)):
            nc.sync.dma_start(t_[:], d_.ap())
        ones16_1 = const_p.tile([NCH, 1], BF16)
        nc.vector.memset(ones16_1[:], 1.0)
        ones1_16 = const_p.tile([1, NCH], BF16)
        nc.vector.memset(ones1_16[:], 1.0)
        eps128 = const_p.tile([P, 1], F32)
        nc.vector.memset(eps128[:], EPS)
        eps16 = const_p.tile([NCH, 1], F32)
        nc.vector.memset(eps16[:], EPS)
        if has_mkb or has_gtb:
            mkb = const_p.tile([1, C], BF16)
            gtb = const_p.tile([1, C], BF16)
            nc.sync.dma_start(mkb[:], mkb_d.ap())
            nc.sync.dma_start(gtb[:], gtb_d.ap())
        if has_pjb:
            pjb = const_p.tile([1, C], BF16)
            nc.sync.dma_start(pjb[:], pjb_d.ap())
        if has_carry_gb:
            cgr = const_p.tile([NCH, D], F32)
            cbr = const_p.tile([NCH, D], F32)
            nc.sync.dma_start(cgr[:], cgr_d.ap())
            nc.sync.dma_start(cbr[:], cbr_d.ap())

        # resident x (transposed), one tile per (chan-group, position-group)
        xt = [[xt_p.tile([P, 512], BF16, tag=f"xt{g}_{pg}",
                         name=f"xt{g}_{pg}") for pg in range(NPG)]
              for g in range(NG)]
        # resident pjw (loaded later; pool allocated at top level)
        pjw_p = top.enter_context(tc.tile_pool(name="pjp", bufs=1))
        pjw = [pjw_p.tile([P, C], BF16, tag=f"pj{k}", name=f"pj{k}")
               for k in range(NG)]
        lgr = pjw_p.tile([P, C], F32) if has_ln_g else None
        lbr = pjw_p.tile([P, C], F32) if has_ln_b else None

        # resident local_cum (bf16) + chunk sums + normalized carries
        lc_sb = []
        for j in range(NCH):
            t_ = lc_p.tile([P, C], BF16, tag=f"lc{j}", name=f"lc{j}")
            lc_sb.append(t_)
        cs_sb = carr_p.tile([NCH, C], BF16)
        ncarry = carr_p.tile([NCH, C], BF16)

        # ================ phase A: pm/gate/scan ================
        with tc.tile_pool(name="wgt", bufs=1) as wgt_p, \
             tc.tile_pool(name="ph1", bufs=2) as ph1_p, \
             tc.tile_pool(name="psA", bufs=1, space="PSUM") as psA_p, \
             tc.tile_pool(name="pslc", bufs=2, space="PSUM") as pslc_p:
            mkw, gtw = [], []
            for k in range(NG):
                mt = wgt_p.tile([P, C], BF16, tag=f"mk{k}", name=f"mk{k}")
                gt_ = wgt_p.tile([P, C], BF16, tag=f"gk{k}", name=f"gk{k}")
                nc.sync.dma_start(mt[:], mkw_d.ap()[k * P:(k + 1) * P, :])
                nc.sync.dma_start(gt_[:], gtw_d.ap()[k * P:(k + 1) * P, :])
                mkw.append(mt)
                gtw.append(gt_)
            # xt in position-group order so compute starts after pg0 lands
            for pg in range(NPG):
                sl = slice(pg * 512, (pg + 1) * 512)
                for g in range(NG):
                    nc.sync.dma_start(xt[g][pg][:],
                                      xt_d.ap()[g * P:(g + 1) * P, sl])
            for k in range(NG):
                nc.sync.dma_start(pjw[k][:], pjw_d.ap()[k * P:(k + 1) * P, :])
            if has_ln_g:
                nc.sync.dma_start(lgr[:], lgr_d.ap())
            if has_ln_b:
                nc.sync.dma_start(lbr[:], lbr_d.ap())
            for j in range(NCH):
                pm0 = psA_p.tile([P, 512], F32, tag="pm0", name="pm0")
                gt0 = psA_p.tile([P, 512], F32, tag="gt0", name="gt0")
                pm1 = psA_p.tile([P, 512], F32, tag="pm1", name="pm1")
                gt1 = psA_p.tile([P, 512], F32, tag="gt1", name="gt1")
                s0, s1_ = slice(0, 512), slice(512, 1024)
                jp, jc = j // 4, (j % 4) * P
                for k in range(NG):
                    lhs = xt[k][jp][:, jc:jc + P]
                    st_ = (k == 0)
                    spm = (k == NG - 1) and not has_mkb
                    spg = (k == NG - 1) and not has_gtb
                    nc.tensor.matmul(pm0[:], lhs, mkw[k][:, s0],
                                     start=st_, stop=spm)
                    nc.tensor.matmul(gt0[:], lhs, gtw[k][:, s0],
                                     start=st_, stop=spg)
                    nc.tensor.matmul(pm1[:], lhs, mkw[k][:, s1_],
                                     start=st_, stop=spm)
                    nc.tensor.matmul(gt1[:], lhs, gtw[k][:, s1_],
                                     start=st_, stop=spg)
                if has_mkb:
                    nc.tensor.matmul(pm0[:], ones1r[:], mkb[:, s0],
                                     start=False, stop=True)
                    nc.tensor.matmul(pm1[:], ones1r[:], mkb[:, s1_],
                                     start=False, stop=True)
                if has_gtb:
                    nc.tensor.matmul(gt0[:], ones1r[:], gtb[:, s0],
                                     start=False, stop=True)
                    nc.tensor.matmul(gt1[:], ones1r[:], gtb[:, s1_],
                                     start=False, stop=True)
                gated = []
                for n, (pm_ps, gt_ps) in enumerate(((pm0, gt0), (pm1, gt1))):
                    gates = ph1_p.tile([P, 512], F32, tag=f"gates{n}",
                                       name=f"gates{n}")
                    nc.scalar.activation(gates[:], gt_ps[:], ACTF.Sigmoid)
                    gd = ph1_p.tile([P, 512], BF16, tag=f"gated{n}",
                                    name=f"gated{n}")
                    nc.vector.tensor_tensor(gd[:], gates[:], pm_ps[:],
                                            op=ALU.mult)
                    gated.append(gd)
                lcps = []
                for n in range(2):
                    lp = pslc_p.tile([P, 512], F32, tag=f"lcps{n}",
                                     name=f"lcps{n}")
                    nc.tensor.matmul(lp[:], ut[:], gated[n][:],
                                     start=True, stop=True)
                    lcps.append(lp)
                for n in range(2):
                    sl = slice(n * 512, (n + 1) * 512)
                    nc.scalar.activation(lc_sb[j][:, sl], lcps[n][:],
                                         ACTF.Copy)
                    nc.sync.dma_start(cs_sb[j:j + 1, sl],
                                      lc_sb[j][127:128, sl])

        # ================ carries + collective ================
        with tc.tile_pool(name="car", bufs=1) as car_p, \
             tc.tile_pool(name="pscar", bufs=1, space="PSUM") as pscar_p:
            tot_ps = pscar_p.tile([1, C], F32, tag="tot")
            carx_ps = pscar_p.tile([NCH, C], F32, tag="carx")
            for n in range(2):
                sl = slice(n * 512, (n + 1) * 512)
                nc.tensor.matmul(tot_ps[:, sl], ones16_1[:], cs_sb[:, sl],
                                 start=True, stop=True)
            ccin_sb = car_p.tile([1, C], F32)
            nc.vector.tensor_scalar(ccin_sb[:], tot_ps[:], segm[:], None,
                                    op0=ALU.mult)
            nc.sync.dma_start(cc_in.ap(), ccin_sb[:])
            nc.gpsimd.collective_compute(
                "AllReduce", ALU.add, replica_groups=groups,
                ins=[cc_in.ap()], outs=[cc_out.ap()])
            # local prefix part runs while the collective is in flight
            for n in range(2):
                sl = slice(n * 512, (n + 1) * 512)
                nc.tensor.matmul(carx_ps[:, sl], l0[:], cs_sb[:, sl],
                                 start=True, stop=False)
            base_sb = car_p.tile([1, C], F32)
            nc.sync.dma_start(base_sb[:], cc_out.ap())
            basem = car_p.tile([1, C], BF16)
            nc.vector.tensor_scalar(basem[:], base_sb[:], usem[:], None,
                                    op0=ALU.mult)
            for n in range(2):
                sl = slice(n * 512, (n + 1) * 512)
                nc.tensor.matmul(carx_ps[:, sl], ones1_16[:],
                                 basem[:, sl], start=False, stop=True)

            # ncarry = LN(carries) over d segments
            c3 = carx_ps[:].rearrange("p (h d) -> p h d", d=D)
            r1 = car_p.tile([NCH, H], F32)
            nc.vector.tensor_reduce(r1[:], c3, axis=AX.X, op=ALU.add)
            sqc = car_p.tile([NCH, C], F32)
            nc.scalar.square(sqc[:], carx_ps[:])
            r2 = car_p.tile([NCH, H], F32)
            nc.vector.tensor_reduce(r2[:], sqc[:].rearrange(
                "p (h d) -> p h d", d=D), axis=AX.X, op=ALU.add)
            mu = car_p.tile([NCH, H], F32)
            nc.vector.tensor_scalar(mu[:], r1[:], 1.0 / D, None, op0=ALU.mult)
            em2 = car_p.tile([NCH, H], F32)
            nc.vector.tensor_scalar(em2[:], r2[:], 1.0 / D, None,
                                    op0=ALU.mult)
            musq = car_p.tile([NCH, H], F32)
            nc.vector.tensor_tensor(musq[:], mu[:], mu[:], op=ALU.mult)
            var = car_p.tile([NCH, H], F32)
            nc.vector.tensor_tensor(var[:], em2[:], musq[:], op=ALU.subtract)
            sd = car_p.tile([NCH, H], F32)
            nc.scalar.activation(sd[:], var[:], ACTF.Sqrt, bias=eps16[:])
            rstd = car_p.tile([NCH, H], F32)
            nc.vector.reciprocal(rstd[:], sd[:])
            mu_b = mu[:].unsqueeze(2).to_broadcast([NCH, H, D])
            rstd_b = rstd[:].unsqueeze(2).to_broadcast([NCH, H, D])
            cen = car_p.tile([NCH, C], F32)
            nc.vector.tensor_tensor(cen[:].rearrange("p (h d) -> p h d", d=D),
                                    c3, mu_b, op=ALU.subtract)
            if has_carry_gb:
                nrm = car_p.tile([NCH, C], F32)
                nc.vector.tensor_tensor(
                    nrm[:].rearrange("p (h d) -> p h d", d=D),
                    cen[:].rearrange("p (h d) -> p h d", d=D), rstd_b,
                    op=ALU.mult)
                cg_b = cgr[:].unsqueeze(1).to_broadcast([NCH, H, D])
                cb_b = cbr[:].unsqueeze(1).to_broadcast([NCH, H, D])
                nrm2 = car_p.tile([NCH, C], F32)
                nc.vector.tensor_tensor(
                    nrm2[:].rearrange("p (h d) -> p h d", d=D),
                    nrm[:].rearrange("p (h d) -> p h d", d=D), cg_b,
                    op=ALU.mult)
                nc.vector.tensor_tensor(
                    ncarry[:].rearrange("p (h d) -> p h d", d=D),
                    nrm2[:].rearrange("p (h d) -> p h d", d=D), cb_b,
                    op=ALU.add)
            else:
                nc.vector.tensor_tensor(
                    ncarry[:].rearrange("p (h d) -> p h d", d=D),
                    cen[:].rearrange("p (h d) -> p h d", d=D), rstd_b,
                    op=ALU.mult)

        # ===== phases C/D/E, software-pipelined per position group =====
        with ExitStack() as late:
            ctp = late.enter_context(tc.tile_pool(name="cardsT", bufs=2))
            otp = late.enter_context(tc.tile_pool(name="outT", bufs=2))
            pc_p = late.enter_context(tc.tile_pool(name="phC", bufs=2))
            pd_p = late.enter_context(tc.tile_pool(name="phD", bufs=2))
            hf_p = late.enter_context(tc.tile_pool(name="hfp", bufs=4))
            pe_p = late.enter_context(tc.tile_pool(name="phE", bufs=2))
            pscl_p = late.enter_context(
                tc.tile_pool(name="pscl", bufs=2, space="PSUM"))
            psh1_p = late.enter_context(
                tc.tile_pool(name="psh1", bufs=2, space="PSUM"))
            psy_p = late.enter_context(
                tc.tile_pool(name="psy", bufs=2, space="PSUM"))

            def phase_C(pg):
                cardsT = [ctp.tile([P, 512], BF16, tag=f"ct{g}",
                                   name=f"ct{pg}_{g}") for g in range(NG)]
                for jj in range(4):
                    j = pg * 4 + jj
                    cl = []
                    for n in range(2):
                        sl = slice(n * 512, (n + 1) * 512)
                        cp = pscl_p.tile([P, 512], F32, tag=f"cl{n}",
                                         name=f"cl{n}")
                        nc.tensor.matmul(cp[:], st[:], lc_sb[j][:, sl],
                                         start=True, stop=False)
                        cl.append(cp)
                    for n in range(2):
                        sl = slice(n * 512, (n + 1) * 512)
                        nc.tensor.matmul(cl[n][:],
                                         selb[:, j * P:(j + 1) * P],
                                         ncarry[:, sl],
                                         start=False, stop=True)
                    cards = pc_p.tile([P, C], BF16, tag="cards",
                                      name=f"cards{j}")
                    for n in range(2):
                        cln = cl[n]
                        cl3 = cln[:].rearrange("p (h d) -> p h d", d=D)
                        sq = pc_p.tile([P, 512], F32, tag=f"sq{n}",
                                       name=f"sq{n}", bufs=1)
                        nc.scalar.square(sq[:], cln[:])
                        r1c = pc_p.tile([P, HH], F32, tag=f"r1c{n}",
                                        name=f"r1c{n}")
                        nc.vector.tensor_reduce(r1c[:], cl3, axis=AX.X,
                                                op=ALU.add)
                        r2c = pc_p.tile([P, HH], F32, tag=f"r2c{n}",
                                        name=f"r2c{n}")
                        nc.vector.tensor_reduce(
                            r2c[:], sq[:].rearrange("p (h d) -> p h d", d=D),
                            axis=AX.X, op=ALU.add)
                        muc = pc_p.tile([P, HH], F32, tag=f"muc{n}",
                                        name=f"muc{n}")
                        nc.vector.tensor_scalar(muc[:], r1c[:], 1.0 / D,
                                                None, op0=ALU.mult)
                        em2c = pc_p.tile([P, HH], F32, tag=f"em2c{n}",
                                         name=f"em2c{n}")
                        nc.vector.tensor_scalar(em2c[:], r2c[:], 1.0 / D,
                                                None, op0=ALU.mult)
                        musqc = pc_p.tile([P, HH], F32, tag=f"musqc{n}",
                                          name=f"musqc{n}")
                        nc.vector.tensor_tensor(musqc[:], muc[:], muc[:],
                                                op=ALU.mult)
                        varc = pc_p.tile([P, HH], F32, tag=f"varc{n}",
                                         name=f"varc{n}")
                        nc.vector.tensor_tensor(varc[:], em2c[:], musqc[:],
                                                op=ALU.subtract)
                        sdc = pc_p.tile([P, HH], F32, tag=f"sdc{n}",
                                        name=f"sdc{n}")
                        nc.scalar.activation(sdc[:], varc[:], ACTF.Sqrt,
                                             bias=eps128[:])
                        rstdc = pc_p.tile([P, HH], F32, tag=f"rstdc{n}",
                                          name=f"rstdc{n}")
                        nc.vector.reciprocal(rstdc[:], sdc[:])
                        mu_bc = muc[:].unsqueeze(2).to_broadcast([P, HH, D])
                        rstd_bc = rstdc[:].unsqueeze(2).to_broadcast(
                            [P, HH, D])
                        cenc = pc_p.tile([P, 512], F32, tag=f"cenc{n}",
                                         name=f"cenc{n}", bufs=1)
                        nc.vector.tensor_tensor(
                            cenc[:].rearrange("p (h d) -> p h d", d=D),
                            cl3, mu_bc, op=ALU.subtract)
                        sl = slice(n * 512, (n + 1) * 512)
                        nc.vector.tensor_tensor(
                            cards[:, sl].rearrange("p (h d) -> p h d", d=D),
                            cenc[:].rearrange("p (h d) -> p h d", d=D),
                            rstd_bc, op=ALU.mult)
                    for g in range(NG):
                        nc.sync.dma_start_transpose(
                            cardsT[g][:, jj * P:(jj + 1) * P],
                            cards[:, g * P:(g + 1) * P])
                return cardsT

            def phase_D(pg, cardsT):
                outT = [otp.tile([P, 512], BF16, tag=f"ot{g}",
                                 name=f"ot{pg}_{g}") for g in range(NG)]
                hfs = []
                # head pairs sharing a stationary: (0,2),(4,6),... then odd
                order = [(off, q) for off in range(2) for q in range(4)]
                for off, q in order:
                    h_a, h_b = off + 4 * q, off + 4 * q + 2
                    o = off * D
                    hps = {}
                    for h in (h_a, h_b):
                        g = h // 2
                        hp = psh1_p.tile([P, 512], F32, tag="h1",
                                         name=f"h1_{pg}_{h}")
                        nc.tensor.matmul(hp[:], w1x[o:o + D, :],
                                         xt[g][pg][o:o + D, :],
                                         start=True, stop=False)
                        hps[h] = hp
                    for h in (h_a, h_b):
                        g = h // 2
                        nc.tensor.matmul(hps[h][:], w1c[o:o + D, :],
                                         cardsT[g][o:o + D, :],
                                         start=False, stop=True)
                    # evacuate h1 early: hb = h1 + b1 (scalar Copy w/ bias)
                    hbs = {}
                    for h in (h_a, h_b):
                        hb = pd_p.tile([P, 512], F32, tag=f"hb{h % 2}",
                                       name=f"hb_{h}")
                        nc.scalar.activation(hb[:], hps[h][:], ACTF.Identity,
                                             bias=b1c[:])
                        hbs[h] = hb
                    sqs = {}
                    for h in (h_a, h_b):
                        sq3 = pd_p.tile([P, 512], F32, tag=f"sq3{h % 2}",
                                        name=f"sq3_{h}", bufs=1)
                        nc.scalar.activation(sq3[:], hbs[h][:], ACTF.Square)
                        sqs[h] = sq3
                    e3s = {}
                    for h in (h_a, h_b):
                        e3 = pd_p.tile([P, 512], F32, tag=f"e3{h % 2}",
                                       name=f"e3_{h}", bufs=1)
                        nc.scalar.activation(e3[:], sqs[h][:], ACTF.Exp,
                                             scale=-0.5)
                        e3s[h] = e3
                    for h in (h_a, h_b):
                        u = pd_p.tile([P, 512], F32, tag=f"u{h % 2}",
                                      name=f"u_{h}", bufs=1)
                        nc.gpsimd.tensor_tensor(u[:], hbs[h][:], e3s[h][:],
                                                op=ALU.mult)
                        hf = hf_p.tile([P, 512], BF16, tag=f"hf{h}",
                                       name=f"hf_{pg}_{h}", bufs=1)
                        nc.vector.scalar_tensor_tensor(
                            hf[:], u[:], float(alpha), hbs[h][:],
                            op0=ALU.mult, op1=ALU.add)
                        hfs.append((h, hf))
                for i, (h, hf) in enumerate(hfs):
                    g, o = h // 2, (h % 2) * D
                    op = psh1_p.tile([P, 512], F32, tag="h1", name=f"o2_{h}")
                    nc.tensor.matmul(op[0:D, :], w2[:], hf[:],
                                     start=True, stop=True)
                    if i % 2 == 0:
                        nc.vector.tensor_copy(outT[g][o:o + D, :],
                                              op[0:D, :])
                    else:
                        nc.scalar.copy(outT[g][o:o + D, :], op[0:D, :])
                return outT

            def phase_E(pg, outT):
                for tt in range(4):
                    t_i = pg * 4 + tt
                    col = tt * P
                    xa = pe_p.tile([P, C], F32, tag="xa", name=f"xa{t_i}",
                                   bufs=1)
                    nc.sync.dma_start(xa[:],
                                      xn_d.ap()[t_i * P:(t_i + 1) * P, :])
                    yp = []
                    for n in range(2):
                        yp.append(psy_p.tile([P, 512], F32, tag="yps",
                                             name=f"yps{t_i}_{n}"))
                    for k in range(NG):
                        lhs = outT[k][:, col:col + P]
                        st_ = (k == 0)
                        sp = (k == NG - 1) and not has_pjb
                        for n in range(2):
                            sl = slice(n * 512, (n + 1) * 512)
                            nc.tensor.matmul(yp[n][:], lhs, pjw[k][:, sl],
                                             start=st_, stop=sp)
                    if has_pjb:
                        for n in range(2):
                            sl = slice(n * 512, (n + 1) * 512)
                            nc.tensor.matmul(yp[n][:], ones1r[:],
                                             pjb[:, sl],
                                             start=False, stop=True)
                    yraw, s1h, s2h = [], [], []
                    for n in range(2):
                        yr = pe_p.tile([P, 512], F32, tag=f"yraw{n}",
                                       name=f"yraw{t_i}_{n}")
                        s1n = pe_p.tile([P, 1], F32, tag=f"s1{n}",
                                        name=f"s1_{t_i}_{n}")
                        nc.scalar.activation(yr[:], yp[n][:], ACTF.Copy,
                                             accum_out=s1n[:])
                        yraw.append(yr)
                        s1h.append(s1n)
                    for n in range(2):
                        sc4 = pe_p.tile([P, 512], F32, tag="sc4",
                                        name=f"sc4_{t_i}_{n}", bufs=1)
                        s2n = pe_p.tile([P, 1], F32, tag=f"s2{n}",
                                        name=f"s2_{t_i}_{n}")
                        nc.scalar.activation(sc4[:], yraw[n][:], ACTF.Square,
                                             scale=1.0 / 32.0,
                                             accum_out=s2n[:])
                        s2h.append(s2n)
                    s1t = pe_p.tile([P, 1], F32, tag="s1t", name=f"s1t{t_i}")
                    nc.vector.tensor_tensor(s1t[:], s1h[0][:], s1h[1][:],
                                            op=ALU.add)
                    m1 = pe_p.tile([P, 1], F32, tag="m1", name=f"m1_{t_i}")
                    nc.vector.tensor_scalar(m1[:], s1t[:], 1.0 / C, None,
                                            op0=ALU.mult)
                    s2t = pe_p.tile([P, 1], F32, tag="s2t", name=f"s2t{t_i}")
                    nc.vector.tensor_tensor(s2t[:], s2h[0][:], s2h[1][:],
                                            op=ALU.add)
                    msq = pe_p.tile([P, 1], F32, tag="msq", name=f"msq{t_i}")
                    nc.vector.tensor_tensor(msq[:], m1[:], m1[:],
                                            op=ALU.mult)
                    var4 = pe_p.tile([P, 1], F32, tag="var4",
                                     name=f"var4_{t_i}")
                    nc.vector.tensor_tensor(var4[:], s2t[:], msq[:],
                                            op=ALU.subtract)
                    sd4 = pe_p.tile([P, 1], F32, tag="sd4",
                                    name=f"sd4_{t_i}")
                    nc.scalar.activation(sd4[:], var4[:], ACTF.Sqrt,
                                         bias=eps128[:])
                    rstd4 = pe_p.tile([P, 1], F32, tag="rstd4",
                                      name=f"rstd4_{t_i}")
                    nc.vector.reciprocal(rstd4[:], sd4[:])
                    yout = pe_p.tile([P, C], F32, tag="yout",
                                     name=f"yout{t_i}")
                    for n in range(2):
                        sl = slice(n * 512, (n + 1) * 512)
                        tn = pe_p.tile([P, 512], F32, tag=f"tn{n}",
                                       name=f"tn{t_i}_{n}", bufs=1)
                        nc.vector.tensor_scalar(tn[:], yraw[n][:], m1[:],
                                                rstd4[:], op0=ALU.subtract,
                                                op1=ALU.mult)
                        if has_ln_g:
                            nc.vector.tensor_tensor(tn[:], tn[:], lgr[:, sl],
                                                    op=ALU.mult)
                        if has_ln_b:
                            nc.vector.tensor_tensor(tn[:], tn[:], lbr[:, sl],
                                                    op=ALU.add)
                        nc.gpsimd.tensor_tensor(yout[:, sl], tn[:],
                                                xa[:, sl], op=ALU.add)
                    nc.sync.dma_start(y_d.ap()[t_i * P:(t_i + 1) * P, :],
                                      yout[:])

            cardsT = {}
            outT = {}
            for pg in range(NPG):
                cardsT[pg] = phase_C(pg)
                if pg >= 1:
                    outT[pg - 1] = phase_D(pg - 1, cardsT.pop(pg - 1))
                    phase_E(pg - 1, outT.pop(pg - 1))
            outT[NPG - 1] = phase_D(NPG - 1, cardsT.pop(NPG - 1))
            phase_E(NPG - 1, outT.pop(NPG - 1))

    nc.compile()
    return nc


_CACHE = {}


def _get_program(alpha, flags):
    key = (alpha, flags)
    if key not in _CACHE:
        _CACHE[key] = _build(NCORES, alpha, *flags)
    return _CACHE[key]


def _bf16(a):
    return np.ascontiguousarray(a.astype(ml_dtypes.bfloat16))


def prepare(inputs):
    """Compute flags + the per-core input maps (host-side prep)."""
    x = np.ascontiguousarray(np.asarray(inputs["x"], np.float32))
    mark_W = np.asarray(inputs["mark_W"], np.float32)
    mark_b = np.asarray(inputs["mark_b"], np.float32)
    gate_W = np.asarray(inputs["gate_W"], np.float32)
    gate_b = np.asarray(inputs["gate_b"], np.float32)
    carry_g = np.asarray(inputs["carry_g"], np.float32)
    carry_b = np.asarray(inputs["carry_b"], np.float32)
    card_g = np.asarray(inputs["card_g"], np.float32)
    card_b = np.asarray(inputs["card_b"], np.float32)
    W1 = np.asarray(inputs["W1"], np.float32)
    b1 = np.asarray(inputs["b1"], np.float32)
    alpha = float(np.asarray(inputs["alpha"]))
    W2 = np.asarray(inputs["W2"], np.float32)
    b2 = np.asarray(inputs["b2"], np.float32)
    proj_W = np.asarray(inputs["proj_W"], np.float32)
    proj_b = np.asarray(inputs["proj_b"], np.float32)
    ln_g = np.asarray(inputs["ln_g"], np.float32)
    ln_b = np.asarray(inputs["ln_b"], np.float32)

    W1x = np.concatenate([W1[:D, :], W1[:D, :]], 0)
    W1c0 = card_g[:, None] * W1[D:, :]
    W1c = np.concatenate([W1c0, W1c0], 0)
    b1f = (b1 + card_b @ W1[D:, :]).astype(np.float32)
    # fold b2 into the proj bias: y = ho @ proj_W + proj_b, ho = .. + b2
    row = np.tile(b2, H).astype(np.float32)
    pjb_eff = (proj_b + row @ proj_W).astype(np.float32)

    stm = np.zeros((P, P), np.float32)
    for i in range(1, P):
        stm[i - 1, i] = 1.0

    has_carry_gb = bool(np.any(carry_g != 1.0) or np.any(carry_b != 0.0))
    flags = (bool(np.any(mark_b)), bool(np.any(gate_b)),
             bool(np.any(pjb_eff)), has_carry_gb,
             bool(np.any(ln_g != 1.0)), bool(np.any(ln_b)))

    common = {
        "mkw": _bf16(mark_W), "gtw": _bf16(gate_W), "pjw": _bf16(proj_W),
        "mkb": _bf16(mark_b[None, :]), "gtb": _bf16(gate_b[None, :]),
        "pjb": _bf16(pjb_eff[None, :]),
        "w1x": _bf16(W1x), "w1c": _bf16(W1c), "b1c": b1f[:, None],
        "w2": _bf16(W2),
        "ut": _bf16(np.triu(np.ones((P, P), np.float32))),
        "st": np.ascontiguousarray(stm.astype(ml_dtypes.bfloat16)),
        "l0": _bf16(np.triu(np.ones((NCH, NCH), np.float32), k=1)),
        "onesr": _bf16(np.ones((1, P), np.float32)),
        "selb": _bf16(np.concatenate(
            [np.eye(NCH, dtype=np.float32)[:, j:j + 1] * np.ones((1, P))
             for j in range(NCH)], axis=1)),
        "cgr": np.tile(carry_g[None, :], (NCH, 1)).astype(np.float32),
        "cbr": np.tile(carry_b[None, :], (NCH, 1)).astype(np.float32),
        "lgr": np.tile(ln_g[None, :], (P, 1)).astype(np.float32),
        "lbr": np.tile(ln_b[None, :], (P, 1)).astype(np.float32),
    }
    in_maps = []
    for c in range(NCORES):
        b, half = c // 2, c % 2
        xs = x[b, half * R:(half + 1) * R, :]
        m = dict(common)
        m["xn"] = np.ascontiguousarray(xs)
        m["xt"] = _bf16(xs.T)
        m["segm"] = np.array([[1.0 - half]], np.float32)
        m["usem"] = np.array([[float(half)]], np.float32)
        in_maps.append(m)
    return alpha, flags, in_maps


def kernel(**inputs):
    alpha, flags, in_maps = prepare(inputs)
    nc = _get_program(alpha, flags)
    res = run_bass_kernel_spmd(nc, in_maps, list(range(NCORES)))
    out = np.empty((B, T, C), np.float32)
    for c in range(NCORES):
        b, half = c // 2, c % 2
        out[b, half * R:(half + 1) * R, :] = res.results[c]["y"]
    return out


# revision 50
# speedup vs baseline: 1.1911x; 1.1717x over previous
"""Trainium2 Bass kernel for nn_ChunkedMultiHeadCardPassingLayer.

Sharding: 8 cores = (batch b = core//2) x (T-half = core%2). Each core
processes 2048 contiguous tokens of one batch end-to-end; the only
cross-core dependency is the chunk-carry prefix, resolved with a 4KB
paired AllReduce.

v2 restructure vs baseline:
- all matmul stationaries are 2-byte (bf16) -> cheap LDWEIGHTS
- local_cum kept in SBUF as bf16 (no DRAM spill round-trip)
- chunk sums extracted from cumsum row 127 (csel matmul dropped)
- cards transposed via XBAR DMA-transpose (no PE transposes, no PSUM)
- MLP activation chain spread across scalar/vector/gpsimd engines
- b1/b2 folded into downstream biases; phase pipeline interleaved
"""
import os
os.environ.setdefault("JAX_PLATFORMS", "cpu")

import numpy as np
import ml_dtypes
from contextlib import ExitStack

import concourse.bacc as bacc
import concourse.mybir as mybir
import concourse.tile as tile
from concourse.bass_utils import run_bass_kernel_spmd

F32 = mybir.dt.float32
F32R = mybir.dt.float32r
BF16 = mybir.dt.bfloat16
AX = mybir.AxisListType
ALU = mybir.AluOpType
ACTF = mybir.ActivationFunctionType

# problem constants
B, T, C = 4, 4096, 1024
H, CS = 16, 128
D = C // H            # 64
NCORES = 8
R = T // 2            # 2048 rows per core
NCH = R // CS         # 16 chunks per core
NG = C // 128         # 8 groups of (2 heads x 64)
NPG = NCH // 4        # 4 position groups of 512
EPS = 1e-5
P = 128
HH = 8                # heads per 512 half


def _build(ncores, alpha, has_mkb, has_gtb, has_pjb,
           has_carry_gb, has_ln_g, has_ln_b):
    nc = bacc.Bacc("TRN2", target_bir_lowering=False, debug=False,
                   num_devices=ncores)

    # ---------------- DRAM I/O ----------------
    xt_d = nc.dram_tensor("xt", [C, R], BF16, kind="ExternalInput")
    xn_d = nc.dram_tensor("xn", [R, C], F32, kind="ExternalInput")
    mkw_d = nc.dram_tensor("mkw", [C, C], BF16, kind="ExternalInput")
    gtw_d = nc.dram_tensor("gtw", [C, C], BF16, kind="ExternalInput")
    pjw_d = nc.dram_tensor("pjw", [C, C], BF16, kind="ExternalInput")
    mkb_d = nc.dram_tensor("mkb", [1, C], BF16, kind="ExternalInput")
    gtb_d = nc.dram_tensor("gtb", [1, C], BF16, kind="ExternalInput")
    pjb_d = nc.dram_tensor("pjb", [1, C], BF16, kind="ExternalInput")
    w1x_d = nc.dram_tensor("w1x", [2 * D, 2 * D], BF16, kind="ExternalInput")
    w1c_d = nc.dram_tensor("w1c", [2 * D, 2 * D], BF16, kind="ExternalInput")
    b1_d = nc.dram_tensor("b1c", [2 * D, 1], F32, kind="ExternalInput")
    w2_d = nc.dram_tensor("w2", [2 * D, D], BF16, kind="ExternalInput")
    w2a_d = nc.dram_tensor("w2a", [2 * D, D], BF16, kind="ExternalInput")
    ut_d = nc.dram_tensor("ut", [P, P], BF16, kind="ExternalInput")
    st_d = nc.dram_tensor("st", [P, P], BF16, kind="ExternalInput")
    l0_d = nc.dram_tensor("l0", [NCH, NCH], BF16, kind="ExternalInput")
    onesr_d = nc.dram_tensor("onesr", [1, P], BF16, kind="ExternalInput")
    selb_d = nc.dram_tensor("selb", [NCH, NCH * P], BF16,
                            kind="ExternalInput")
    segm_d = nc.dram_tensor("segm", [1, 1], F32, kind="ExternalInput")
    usem_d = nc.dram_tensor("usem", [1, 1], F32, kind="ExternalInput")
    cgr_d = nc.dram_tensor("cgr", [NCH, D], F32, kind="ExternalInput")
    cbr_d = nc.dram_tensor("cbr", [NCH, D], F32, kind="ExternalInput")
    lgr_d = nc.dram_tensor("lgr", [P, C], F32, kind="ExternalInput")
    lbr_d = nc.dram_tensor("lbr", [P, C], F32, kind="ExternalInput")

    y_d = nc.dram_tensor("y", [R, C], F32, kind="ExternalOutput")

    cc_in = nc.dram_tensor("cc_in", [1, C], F32)
    cc_out = nc.dram_tensor("cc_out", [1, C], F32)

    groups = ([[i, i + 1] for i in range(0, ncores, 2)]
              if ncores > 1 else [[0]])

    with tile.TileContext(nc) as tc, ExitStack() as top:
        const_p = top.enter_context(tc.tile_pool(name="const", bufs=1))
        xt_p = top.enter_context(tc.tile_pool(name="xtp", bufs=1))
        lc_p = top.enter_context(tc.tile_pool(name="lcp", bufs=1))
        carr_p = top.enter_context(tc.tile_pool(name="carr", bufs=1))

        # ---------- constants ----------
        ut = const_p.tile([P, P], BF16)
        st = const_p.tile([P, P], BF16)
        l0 = const_p.tile([NCH, NCH], BF16)
        w1x = const_p.tile([2 * D, 2 * D], BF16)
        w1c = const_p.tile([2 * D, 2 * D], BF16)
        b1c = const_p.tile([2 * D, 1], F32)
        w2 = const_p.tile([2 * D, D], BF16)
        w2a = const_p.tile([2 * D, D], BF16)
        segm = const_p.tile([1, 1], F32)
        usem = const_p.tile([1, 1], F32)
        ones1r = const_p.tile([1, P], BF16)
        selb = const_p.tile([NCH, NCH * P], BF16)
        for t_, d_ in ((ut, ut_d), (st, st_d), (l0, l0_d),
                       (w1x, w1x_d), (w1c, w1c_d), (b1c, b1_d),
                       (w2, w2_d), (w2a, w2a_d), (segm, segm_d),
                       (usem, usem_d), (ones1r, onesr_d), (selb, selb_d)):
            nc.sync.dma_start(t_[:], d_.ap())
        ones16_1 = const_p.tile([NCH, 1], BF16)
        nc.vector.memset(ones16_1[:], 1.0)
        ones1_16 = const_p.tile([1, NCH], BF16)
        nc.vector.memset(ones1_16[:], 1.0)
        eps128 = const_p.tile([P, 1], F32)
        nc.vector.memset(eps128[:], EPS)
        eps16 = const_p.tile([NCH, 1], F32)
        nc.vector.memset(eps16[:], EPS)
        if has_mkb or has_gtb:
            mkb = const_p.tile([1, C], BF16)
            gtb = const_p.tile([1, C], BF16)
            nc.sync.dma_start(mkb[:], mkb_d.ap())
            nc.sync.dma_start(gtb[:], gtb_d.ap())
        if has_pjb:
            pjb = const_p.tile([1, C], BF16)
            nc.sync.dma_start(pjb[:], pjb_d.ap())
        if has_carry_gb:
            cgr = const_p.tile([NCH, D], F32)
            cbr = const_p.tile([NCH, D], F32)
            nc.sync.dma_start(cgr[:], cgr_d.ap())
            nc.sync.dma_start(cbr[:], cbr_d.ap())

        # resident x (transposed), one tile per (chan-group, position-group)
        xt = [[xt_p.tile([P, 512], BF16, tag=f"xt{g}_{pg}",
                         name=f"xt{g}_{pg}") for pg in range(NPG)]
              for g in range(NG)]
        # resident pjw (loaded later; pool allocated at top level)
        pjw_p = top.enter_context(tc.tile_pool(name="pjp", bufs=1))
        pjw = [pjw_p.tile([P, C], BF16, tag=f"pj{k}", name=f"pj{k}")
               for k in range(NG)]
        lgr = pjw_p.tile([P, C], F32) if has_ln_g else None
        lbr = pjw_p.tile([P, C], F32) if has_ln_b else None

        # resident local_cum (bf16) + chunk sums + normalized carries
        lc_sb = []
        for j in range(NCH):
            t_ = lc_p.tile([P, C], BF16, tag=f"lc{j}", name=f"lc{j}")
            lc_sb.append(t_)
        cs_sb = carr_p.tile([NCH, C], BF16)
        ncarry = carr_p.tile([NCH, C], BF16)

        # ================ phase A: pm/gate/scan ================
        with tc.tile_pool(name="wgt", bufs=1) as wgt_p, \
             tc.tile_pool(name="ph1", bufs=2) as ph1_p, \
             tc.tile_pool(name="psA", bufs=1, space="PSUM") as psA_p, \
             tc.tile_pool(name="pslc", bufs=2, space="PSUM") as pslc_p:
            mkw, gtw = [], []
            for k in range(NG):
                mt = wgt_p.tile([P, C], BF16, tag=f"mk{k}", name=f"mk{k}")
                gt_ = wgt_p.tile([P, C], BF16, tag=f"gk{k}", name=f"gk{k}")
                nc.sync.dma_start(mt[:], mkw_d.ap()[k * P:(k + 1) * P, :])
                nc.sync.dma_start(gt_[:], gtw_d.ap()[k * P:(k + 1) * P, :])
                mkw.append(mt)
                gtw.append(gt_)
            # xt in position-group order so compute starts after pg0 lands
            for pg in range(NPG):
                sl = slice(pg * 512, (pg + 1) * 512)
                for g in range(NG):
                    nc.sync.dma_start(xt[g][pg][:],
                                      xt_d.ap()[g * P:(g + 1) * P, sl])
            for k in range(NG):
                nc.sync.dma_start(pjw[k][:], pjw_d.ap()[k * P:(k + 1) * P, :])
            if has_ln_g:
                nc.sync.dma_start(lgr[:], lgr_d.ap())
            if has_ln_b:
                nc.sync.dma_start(lbr[:], lbr_d.ap())
            for j in range(NCH):
                pm0 = psA_p.tile([P, 512], F32, tag="pm0", name="pm0")
                gt0 = psA_p.tile([P, 512], F32, tag="gt0", name="gt0")
                pm1 = psA_p.tile([P, 512], F32, tag="pm1", name="pm1")
                gt1 = psA_p.tile([P, 512], F32, tag="gt1", name="gt1")
                s0, s1_ = slice(0, 512), slice(512, 1024)
                jp, jc = j // 4, (j % 4) * P
                for k in range(NG):
                    lhs = xt[k][jp][:, jc:jc + P]
                    st_ = (k == 0)
                    spm = (k == NG - 1) and not has_mkb
                    spg = (k == NG - 1) and not has_gtb
                    nc.tensor.matmul(pm0[:], lhs, mkw[k][:, s0],
                                     start=st_, stop=spm)
                    nc.tensor.matmul(gt0[:], lhs, gtw[k][:, s0],
                                     start=st_, stop=spg)
                    nc.tensor.matmul(pm1[:], lhs, mkw[k][:, s1_],
                                     start=st_, stop=spm)
                    nc.tensor.matmul(gt1[:], lhs, gtw[k][:, s1_],
                                     start=st_, stop=spg)
                if has_mkb:
                    nc.tensor.matmul(pm0[:], ones1r[:], mkb[:, s0],
                                     start=False, stop=True)
                    nc.tensor.matmul(pm1[:], ones1r[:], mkb[:, s1_],
                                     start=False, stop=True)
                if has_gtb:
                    nc.tensor.matmul(gt0[:], ones1r[:], gtb[:, s0],
                                     start=False, stop=True)
                    nc.tensor.matmul(gt1[:], ones1r[:], gtb[:, s1_],
                                     start=False, stop=True)
                gated = []
                for n, (pm_ps, gt_ps) in enumerate(((pm0, gt0), (pm1, gt1))):
                    gates = ph1_p.tile([P, 512], F32, tag=f"gates{n}",
                                       name=f"gates{n}")
                    nc.scalar.activation(gates[:], gt_ps[:], ACTF.Sigmoid)
                    gd = ph1_p.tile([P, 512], BF16, tag=f"gated{n}",
                                    name=f"gated{n}")
                    nc.vector.tensor_tensor(gd[:], gates[:], pm_ps[:],
                                            op=ALU.mult)
                    gated.append(gd)
                lcps = []
                for n in range(2):
                    lp = pslc_p.tile([P, 512], F32, tag=f"lcps{n}",
                                     name=f"lcps{n}")
                    nc.tensor.matmul(lp[:], ut[:], gated[n][:],
                                     start=True, stop=True)
                    lcps.append(lp)
                for n in range(2):
                    sl = slice(n * 512, (n + 1) * 512)
                    nc.scalar.activation(lc_sb[j][:, sl], lcps[n][:],
                                         ACTF.Copy)
                    nc.sync.dma_start(cs_sb[j:j + 1, sl],
                                      lc_sb[j][127:128, sl])

        # ================ carries + collective ================
        with tc.tile_pool(name="car", bufs=1) as car_p, \
             tc.tile_pool(name="pscar", bufs=1, space="PSUM") as pscar_p:
            tot_ps = pscar_p.tile([1, C], F32, tag="tot")
            carx_ps = pscar_p.tile([NCH, C], F32, tag="carx")
            for n in range(2):
                sl = slice(n * 512, (n + 1) * 512)
                nc.tensor.matmul(tot_ps[:, sl], ones16_1[:], cs_sb[:, sl],
                                 start=True, stop=True)
            ccin_sb = car_p.tile([1, C], F32)
            nc.vector.tensor_scalar(ccin_sb[:], tot_ps[:], segm[:], None,
                                    op0=ALU.mult)
            nc.sync.dma_start(cc_in.ap(), ccin_sb[:])
            nc.gpsimd.collective_compute(
                "AllReduce", ALU.add, replica_groups=groups,
                ins=[cc_in.ap()], outs=[cc_out.ap()])
            # local prefix part runs while the collective is in flight
            for n in range(2):
                sl = slice(n * 512, (n + 1) * 512)
                nc.tensor.matmul(carx_ps[:, sl], l0[:], cs_sb[:, sl],
                                 start=True, stop=False)
            base_sb = car_p.tile([1, C], F32)
            nc.sync.dma_start(base_sb[:], cc_out.ap())
            basem = car_p.tile([1, C], BF16)
            nc.vector.tensor_scalar(basem[:], base_sb[:], usem[:], None,
                                    op0=ALU.mult)
            for n in range(2):
                sl = slice(n * 512, (n + 1) * 512)
                nc.tensor.matmul(carx_ps[:, sl], ones1_16[:],
                                 basem[:, sl], start=False, stop=True)

            # ncarry = LN(carries) over d segments
            c3 = carx_ps[:].rearrange("p (h d) -> p h d", d=D)
            r1 = car_p.tile([NCH, H], F32)
            nc.vector.tensor_reduce(r1[:], c3, axis=AX.X, op=ALU.add)
            sqc = car_p.tile([NCH, C], F32)
            nc.scalar.square(sqc[:], carx_ps[:])
            r2 = car_p.tile([NCH, H], F32)
            nc.vector.tensor_reduce(r2[:], sqc[:].rearrange(
                "p (h d) -> p h d", d=D), axis=AX.X, op=ALU.add)
            mu = car_p.tile([NCH, H], F32)
            nc.vector.tensor_scalar(mu[:], r1[:], 1.0 / D, None, op0=ALU.mult)
            em2 = car_p.tile([NCH, H], F32)
            nc.vector.tensor_scalar(em2[:], r2[:], 1.0 / D, None,
                                    op0=ALU.mult)
            musq = car_p.tile([NCH, H], F32)
            nc.vector.tensor_tensor(musq[:], mu[:], mu[:], op=ALU.mult)
            var = car_p.tile([NCH, H], F32)
            nc.vector.tensor_tensor(var[:], em2[:], musq[:], op=ALU.subtract)
            sd = car_p.tile([NCH, H], F32)
            nc.scalar.activation(sd[:], var[:], ACTF.Sqrt, bias=eps16[:])
            rstd = car_p.tile([NCH, H], F32)
            nc.vector.reciprocal(rstd[:], sd[:])
            mu_b = mu[:].unsqueeze(2).to_broadcast([NCH, H, D])
            rstd_b = rstd[:].unsqueeze(2).to_broadcast([NCH, H, D])
            cen = car_p.tile([NCH, C], F32)
            nc.vector.tensor_tensor(cen[:].rearrange("p (h d) -> p h d", d=D),
                                    c3, mu_b, op=ALU.subtract)
            if has_carry_gb:
                nrm = car_p.tile([NCH, C], F32)
                nc.vector.tensor_tensor(
                    nrm[:].rearrange("p (h d) -> p h d", d=D),
                    cen[:].rearrange("p (h d) -> p h d", d=D), rstd_b,
                    op=ALU.mult)
                cg_b = cgr[:].unsqueeze(1).to_broadcast([NCH, H, D])
                cb_b = cbr[:].unsqueeze(1).to_broadcast([NCH, H, D])
                nrm2 = car_p.tile([NCH, C], F32)
                nc.vector.tensor_tensor(
                    nrm2[:].rearrange("p (h d) -> p h d", d=D),
                    nrm[:].rearrange("p (h d) -> p h d", d=D), cg_b,
                    op=ALU.mult)
                nc.vector.tensor_tensor(
                    ncarry[:].rearrange("p (h d) -> p h d", d=D),
                    nrm2[:].rearrange("p (h d) -> p h d", d=D), cb_b,
                    op=ALU.add)
            else:
                nc.vector.tensor_tensor(
                    ncarry[:].rearrange("p (h d) -> p h d", d=D),
                    cen[:].rearrange("p (h d) -> p h d", d=D), rstd_b,
                    op=ALU.mult)

        # ===== phases C/D/E, software-pipelined per position group =====
        with ExitStack() as late:
            ctp = late.enter_context(tc.tile_pool(name="cardsT", bufs=2))
            otp = late.enter_context(tc.tile_pool(name="outT", bufs=2))
            pc_p = late.enter_context(tc.tile_pool(name="phC", bufs=2))
            pd_p = late.enter_context(tc.tile_pool(name="phD", bufs=2))
            pe_p = late.enter_context(tc.tile_pool(name="phE", bufs=2))
            pscl_p = late.enter_context(
                tc.tile_pool(name="pscl", bufs=2, space="PSUM"))
            psh1_p = late.enter_context(
                tc.tile_pool(name="psh1", bufs=2, space="PSUM"))
            psy_p = late.enter_context(
                tc.tile_pool(name="psy", bufs=2, space="PSUM"))

            def phase_C(pg):
                # transposed cards, blocked layout: block (jj, n, gg) holds
                # chans (4n+gg)*128..+128 on partitions, tokens of chunk
                # pg*4+jj on cols jj*1024 + n*512 + gg*128 ..+128
                ctbig = ctp.tile([P, 4 * C], BF16, tag="ctbig",
                                 name=f"ctbig{pg}")
                for jj in range(4):
                    j = pg * 4 + jj
                    cl = []
                    for n in range(2):
                        sl = slice(n * 512, (n + 1) * 512)
                        cp = pscl_p.tile([P, 512], F32, tag=f"cl{n}",
                                         name=f"cl{n}")
                        nc.tensor.matmul(cp[:], st[:], lc_sb[j][:, sl],
                                         start=True, stop=False)
                        cl.append(cp)
                    for n in range(2):
                        sl = slice(n * 512, (n + 1) * 512)
                        nc.tensor.matmul(cl[n][:],
                                         selb[:, j * P:(j + 1) * P],
                                         ncarry[:, sl],
                                         start=False, stop=True)
                    cards = pc_p.tile([P, C], BF16, tag="cards",
                                      name=f"cards{j}")
                    for n in range(2):
                        cln = cl[n]
                        cl3 = cln[:].rearrange("p (h d) -> p h d", d=D)
                        sq = pc_p.tile([P, 512], F32, tag=f"sq{n}",
                                       name=f"sq{n}", bufs=1)
                        nc.scalar.square(sq[:], cln[:])
                        r1c = pc_p.tile([P, HH], F32, tag=f"r1c{n}",
                                        name=f"r1c{n}")
                        nc.vector.tensor_reduce(r1c[:], cl3, axis=AX.X,
                                                op=ALU.add)
                        r2c = pc_p.tile([P, HH], F32, tag=f"r2c{n}",
                                        name=f"r2c{n}")
                        nc.vector.tensor_reduce(
                            r2c[:], sq[:].rearrange("p (h d) -> p h d", d=D),
                            axis=AX.X, op=ALU.add)
                        muc = pc_p.tile([P, HH], F32, tag=f"muc{n}",
                                        name=f"muc{n}")
                        nc.vector.tensor_scalar(muc[:], r1c[:], 1.0 / D,
                                                None, op0=ALU.mult)
                        em2c = pc_p.tile([P, HH], F32, tag=f"em2c{n}",
                                         name=f"em2c{n}")
                        nc.vector.tensor_scalar(em2c[:], r2c[:], 1.0 / D,
                                                None, op0=ALU.mult)
                        musqc = pc_p.tile([P, HH], F32, tag=f"musqc{n}",
                                          name=f"musqc{n}")
                        nc.vector.tensor_tensor(musqc[:], muc[:], muc[:],
                                                op=ALU.mult)
                        varc = pc_p.tile([P, HH], F32, tag=f"varc{n}",
                                         name=f"varc{n}")
                        nc.vector.tensor_tensor(varc[:], em2c[:], musqc[:],
                                                op=ALU.subtract)
                        sdc = pc_p.tile([P, HH], F32, tag=f"sdc{n}",
                                        name=f"sdc{n}")
                        nc.scalar.activation(sdc[:], varc[:], ACTF.Sqrt,
                                             bias=eps128[:])
                        rstdc = pc_p.tile([P, HH], F32, tag=f"rstdc{n}",
                                          name=f"rstdc{n}")
                        nc.vector.reciprocal(rstdc[:], sdc[:])
                        mu_bc = muc[:].unsqueeze(2).to_broadcast([P, HH, D])
                        rstd_bc = rstdc[:].unsqueeze(2).to_broadcast(
                            [P, HH, D])
                        cenc = pc_p.tile([P, 512], F32, tag=f"cenc{n}",
                                         name=f"cenc{n}", bufs=1)
                        nc.vector.tensor_tensor(
                            cenc[:].rearrange("p (h d) -> p h d", d=D),
                            cl3, mu_bc, op=ALU.subtract)
                        sl = slice(n * 512, (n + 1) * 512)
                        nc.vector.tensor_tensor(
                            cards[:, sl].rearrange("p (h d) -> p h d", d=D),
                            cenc[:].rearrange("p (h d) -> p h d", d=D),
                            rstd_bc, op=ALU.mult)
                    for n in range(2):
                        base = jj * C + n * 512
                        out3 = ctbig[:, base:base + 512].rearrange(
                            "p (b c) -> p b c", c=P)
                        nc.sync.dma_start_transpose(
                            out3, cards[:, n * 512:(n + 1) * 512])

                def cardsT_fn(g, o):
                    n, gg = g // 4, g % 4
                    col = n * 512 + gg * P
                    return ctbig[o:o + D, :].rearrange(
                        "p (jj q) -> p jj q", q=C)[:, :, col:col + P]
                return cardsT_fn

            def phase_D(pg, cardsT):
                outT = [otp.tile([P, 512], BF16, tag=f"ot{g}",
                                 name=f"ot{pg}_{g}") for g in range(NG)]
                # head quads sharing a stationary: (0,2,4,6),(8,..) then odd
                for off, q in [(o_, q_) for o_ in range(2) for q_ in (0, 1)]:
                    quad = [off + 8 * q + 2 * i for i in range(4)]
                    o = off * D
                    hbs, sqs, e3s, us = {}, {}, {}, {}
                    for h in quad:
                        g = h // 2
                        hp = psh1_p.tile([P, 512], F32, tag="h1",
                                         name=f"h1_{pg}_{h}")
                        nc.tensor.matmul(hp[:], w1x[o:o + D, :],
                                         xt[g][pg][o:o + D, :],
                                         start=True, stop=False)
                        nc.tensor.matmul(hp[:], w1c[o:o + D, :],
                                         cardsT(g, o), start=False,
                                         stop=True)
                        # evacuate h1 fast: hb = h1 + b1 (vector, bf16 out)
                        hb = pd_p.tile([P, 512], BF16, tag=f"hb{h % 4}",
                                       name=f"hb_{pg}_{h}")
                        nc.vector.tensor_scalar(hb[:], hp[:], b1c[:], None,
                                                op0=ALU.add)
                        hbs[h] = hb
                    for h in quad:
                        sq3 = pd_p.tile([P, 512], BF16, tag=f"sq3{h % 4}",
                                        name=f"sq3_{h}", bufs=1)
                        nc.gpsimd.tensor_tensor(sq3[:], hbs[h][:],
                                                hbs[h][:], op=ALU.mult)
                        sqs[h] = sq3
                    for h in quad:
                        e3 = pd_p.tile([P, 512], F32, tag=f"e3{h % 4}",
                                       name=f"e3_{h}", bufs=1)
                        nc.scalar.activation(e3[:], sqs[h][:], ACTF.Exp,
                                             scale=-0.5)
                        e3s[h] = e3
                    for h in quad:
                        u = pd_p.tile([P, 512], BF16, tag=f"u{h % 4}",
                                      name=f"u_{h}", bufs=1)
                        nc.vector.tensor_tensor(u[:], hbs[h][:], e3s[h][:],
                                                op=ALU.mult)
                        us[h] = u
                    for i, h in enumerate(quad):
                        g = h // 2
                        op = psh1_p.tile([P, 512], F32, tag="h1",
                                         name=f"o2_{h}")
                        nc.tensor.matmul(op[0:D, :], w2[:], hbs[h][:],
                                         start=True, stop=False)
                        nc.tensor.matmul(op[0:D, :], w2a[:], us[h][:],
                                         start=False, stop=True)
                        if i % 2 == 0:
                            nc.vector.tensor_copy(outT[g][o:o + D, :],
                                                  op[0:D, :])
                        else:
                            nc.scalar.copy(outT[g][o:o + D, :], op[0:D, :])
                return outT

            def phase_E(pg, outT):
                for tt in range(4):
                    t_i = pg * 4 + tt
                    col = tt * P
                    xa = pe_p.tile([P, C], F32, tag="xa", name=f"xa{t_i}",
                                   bufs=1)
                    nc.sync.dma_start(xa[:],
                                      xn_d.ap()[t_i * P:(t_i + 1) * P, :])
                    yp = []
                    for n in range(2):
                        yp.append(psy_p.tile([P, 512], F32, tag="yps",
                                             name=f"yps{t_i}_{n}"))
                    for k in range(NG):
                        lhs = outT[k][:, col:col + P]
                        st_ = (k == 0)
                        sp = (k == NG - 1) and not has_pjb
                        for n in range(2):
                            sl = slice(n * 512, (n + 1) * 512)
                            nc.tensor.matmul(yp[n][:], lhs, pjw[k][:, sl],
                                             start=st_, stop=sp)
                    if has_pjb:
                        for n in range(2):
                            sl = slice(n * 512, (n + 1) * 512)
                            nc.tensor.matmul(yp[n][:], ones1r[:],
                                             pjb[:, sl],
                                             start=False, stop=True)
                    yraw, s1h, s2h = [], [], []
                    for n in range(2):
                        yr = pe_p.tile([P, 512], F32, tag=f"yraw{n}",
                                       name=f"yraw{t_i}_{n}")
                        s1n = pe_p.tile([P, 1], F32, tag=f"s1{n}",
                                        name=f"s1_{t_i}_{n}")
                        nc.scalar.activation(yr[:], yp[n][:], ACTF.Copy,
                                             accum_out=s1n[:])
                        yraw.append(yr)
                        s1h.append(s1n)
                    for n in range(2):
                        sc4 = pe_p.tile([P, 512], F32, tag="sc4",
                                        name=f"sc4_{t_i}_{n}", bufs=1)
                        s2n = pe_p.tile([P, 1], F32, tag=f"s2{n}",
                                        name=f"s2_{t_i}_{n}")
                        nc.scalar.activation(sc4[:], yraw[n][:], ACTF.Square,
                                             scale=1.0 / 32.0,
                                             accum_out=s2n[:])
                        s2h.append(s2n)
                    s1t = pe_p.tile([P, 1], F32, tag="s1t", name=f"s1t{t_i}")
                    nc.vector.tensor_tensor(s1t[:], s1h[0][:], s1h[1][:],
                                            op=ALU.add)
                    m1 = pe_p.tile([P, 1], F32, tag="m1", name=f"m1_{t_i}")
                    nc.vector.tensor_scalar(m1[:], s1t[:], 1.0 / C, None,
                                            op0=ALU.mult)
                    s2t = pe_p.tile([P, 1], F32, tag="s2t", name=f"s2t{t_i}")
                    nc.vector.tensor_tensor(s2t[:], s2h[0][:], s2h[1][:],
                                            op=ALU.add)
                    msq = pe_p.tile([P, 1], F32, tag="msq", name=f"msq{t_i}")
                    nc.vector.tensor_tensor(msq[:], m1[:], m1[:],
                                            op=ALU.mult)
                    var4 = pe_p.tile([P, 1], F32, tag="var4",
                                     name=f"var4_{t_i}")
                    nc.vector.tensor_tensor(var4[:], s2t[:], msq[:],
                                            op=ALU.subtract)
                    sd4 = pe_p.tile([P, 1], F32, tag="sd4",
                                    name=f"sd4_{t_i}")
                    nc.scalar.activation(sd4[:], var4[:], ACTF.Sqrt,
                                         bias=eps128[:])
                    rstd4 = pe_p.tile([P, 1], F32, tag="rstd4",
                                      name=f"rstd4_{t_i}")
                    nc.vector.reciprocal(rstd4[:], sd4[:])
                    yout = pe_p.tile([P, C], F32, tag="yout",
                                     name=f"yout{t_i}")
                    for n in range(2):
                        sl = slice(n * 512, (n + 1) * 512)
                        tn = pe_p.tile([P, 512], F32, tag=f"tn{n}",
                                       name=f"tn{t_i}_{n}", bufs=1)
                        nc.vector.tensor_scalar(tn[:], yraw[n][:], m1[:],
                                                rstd4[:], op0=ALU.subtract,
                                                op1=ALU.mult)
                        if has_ln_g:
                            nc.vector.tensor_tensor(tn[:], tn[:], lgr[:, sl],
                                                    op=ALU.mult)
                        if has_ln_b:
                            nc.vector.tensor_tensor(tn[:], tn[:], lbr[:, sl],
                                                    op=ALU.add)
                        nc.gpsimd.tensor_tensor(yout[:, sl], tn[:],
                                                xa[:, sl], op=ALU.add)
                    nc.sync.dma_start(y_d.ap()[t_i * P:(t_i + 1) * P, :],
                                      yout[:])

            cardsT = {}
            outT = {}
            for pg in range(NPG):
                cardsT[pg] = phase_C(pg)
                if pg >= 1:
                    outT[pg - 1] = phase_D(pg - 1, cardsT.pop(pg - 1))
                    phase_E(pg - 1, outT.pop(pg - 1))
            outT[NPG - 1] = phase_D(NPG - 1, cardsT.pop(NPG - 1))
            phase_E(NPG - 1, outT.pop(NPG - 1))

    nc.compile()
    return nc


_CACHE = {}


def _get_program(alpha, flags):
    key = (alpha, flags)
    if key not in _CACHE:
        _CACHE[key] = _build(NCORES, alpha, *flags)
    return _CACHE[key]


def _bf16(a):
    return np.ascontiguousarray(a.astype(ml_dtypes.bfloat16))


def prepare(inputs):
    """Compute flags + the per-core input maps (host-side prep)."""
    x = np.ascontiguousarray(np.asarray(inputs["x"], np.float32))
    mark_W = np.asarray(inputs["mark_W"], np.float32)
    mark_b = np.asarray(inputs["mark_b"], np.float32)
    gate_W = np.asarray(inputs["gate_W"], np.float32)
    gate_b = np.asarray(inputs["gate_b"], np.float32)
    carry_g = np.asarray(inputs["carry_g"], np.float32)
    carry_b = np.asarray(inputs["carry_b"], np.float32)
    card_g = np.asarray(inputs["card_g"], np.float32)
    card_b = np.asarray(inputs["card_b"], np.float32)
    W1 = np.asarray(inputs["W1"], np.float32)
    b1 = np.asarray(inputs["b1"], np.float32)
    alpha = float(np.asarray(inputs["alpha"]))
    W2 = np.asarray(inputs["W2"], np.float32)
    b2 = np.asarray(inputs["b2"], np.float32)
    proj_W = np.asarray(inputs["proj_W"], np.float32)
    proj_b = np.asarray(inputs["proj_b"], np.float32)
    ln_g = np.asarray(inputs["ln_g"], np.float32)
    ln_b = np.asarray(inputs["ln_b"], np.float32)

    W1x = np.concatenate([W1[:D, :], W1[:D, :]], 0)
    W1c0 = card_g[:, None] * W1[D:, :]
    W1c = np.concatenate([W1c0, W1c0], 0)
    b1f = (b1 + card_b @ W1[D:, :]).astype(np.float32)
    # fold b2 into the proj bias: y = ho @ proj_W + proj_b, ho = .. + b2
    row = np.tile(b2, H).astype(np.float32)
    pjb_eff = (proj_b + row @ proj_W).astype(np.float32)

    stm = np.zeros((P, P), np.float32)
    for i in range(1, P):
        stm[i - 1, i] = 1.0

    has_carry_gb = bool(np.any(carry_g != 1.0) or np.any(carry_b != 0.0))
    flags = (bool(np.any(mark_b)), bool(np.any(gate_b)),
             bool(np.any(pjb_eff)), has_carry_gb,
             bool(np.any(ln_g != 1.0)), bool(np.any(ln_b)))

    common = {
        "mkw": _bf16(mark_W), "gtw": _bf16(gate_W), "pjw": _bf16(proj_W),
        "mkb": _bf16(mark_b[None, :]), "gtb": _bf16(gate_b[None, :]),
        "pjb": _bf16(pjb_eff[None, :]),
        "w1x": _bf16(W1x), "w1c": _bf16(W1c), "b1c": b1f[:, None],
        "w2": _bf16(W2), "w2a": _bf16(alpha * W2),
        "ut": _bf16(np.triu(np.ones((P, P), np.float32))),
        "st": np.ascontiguousarray(stm.astype(ml_dtypes.bfloat16)),
        "l0": _bf16(np.triu(np.ones((NCH, NCH), np.float32), k=1)),
        "onesr": _bf16(np.ones((1, P), np.float32)),
        "selb": _bf16(np.concatenate(
            [np.eye(NCH, dtype=np.float32)[:, j:j + 1] * np.ones((1, P))
             for j in range(NCH)], axis=1)),
        "cgr": np.tile(carry_g[None, :], (NCH, 1)).astype(np.float32),
        "cbr": np.tile(carry_b[None, :], (NCH, 1)).astype(np.float32),
        "lgr": np.tile(ln_g[None, :], (P, 1)).astype(np.float32),
        "lbr": np.tile(ln_b[None, :], (P, 1)).astype(np.float32),
    }
    in_maps = []
    for c in range(NCORES):
        b, half = c // 2, c % 2
        xs = x[b, half * R:(half + 1) * R, :]
        m = dict(common)
        m["xn"] = np.ascontiguousarray(xs)
        m["xt"] = _bf16(xs.T)
        m["segm"] = np.array([[1.0 - half]], np.float32)
        m["usem"] = np.array([[float(half)]], np.float32)
        in_maps.append(m)
    return alpha, flags, in_maps


def kernel(**inputs):
    alpha, flags, in_maps = prepare(inputs)
    nc = _get_program(alpha, flags)
    res = run_bass_kernel_spmd(nc, in_maps, list(range(NCORES)))
    out = np.empty((B, T, C), np.float32)
    for c in range(NCORES):
        b, half = c // 2, c % 2
        out[b, half * R:(half + 1) * R, :] = res.results[c]["y"]
    return out
